# revision 1
# baseline (speedup 1.0000x reference)
"""Trainium2 Bass kernel for 12-head causal MHA (B=2, S=2048, D=768), fp32.

Sharding: 8 cores = (batch b in {0,1}) x (head-group hg in {0..3}, 3 heads each).
Each core computes, for its (b, hg):
    qT/kT = (x wq_hg^T)^T  (transposed layout, [192, S])
    v     = x wv_hg^T      (natural layout, + ones column for softmax denom)
    flash-style causal attention without max-subtraction (scores are O(1))
    partial yT = wo_hg^T @ outT   ([768, S], row-parallel partial)
Host sums the 4 head-group partials per batch, transposes, adds bo.

Matmul operands live in SBUF as float32r (fp32 bits; 1 cycle/row at N>=256).
The causal mask is a multiplicative 0/1 mask sliced from a [128, 1024]
sliding-window matrix (host input), applied only on diagonal-band tiles.
"""

import math
from contextlib import ExitStack

import numpy as np

import concourse.bacc as bacc
import concourse.bass as bass
import concourse.mybir as mybir
import concourse.tile as tile

FP32 = mybir.dt.float32
FP32R = mybir.dt.float32r

B = 2
S = 2048
D = 768
NH = 12
DK = 64
NCORES = 8
HG = 3  # heads per core
HD = HG * DK  # 192
VP = 256  # padded v width (>=256 keeps float32r at full rate)
KC = D // 128  # 6 contraction chunks of 128
SB = 512  # sequence block (matmul N)
NJ = S // SB  # 4
NT = S // 128  # 16 key tiles
SCALE = 1.0 / math.sqrt(DK)
PSUM_BUFS = (2, 2, 2, 2)  # proj, sp (scores), op (attn out), yp (y proj)


def _r(ap):
    """float32r view of an fp32 DRAM AP (same bytes) for DMA into fp32r SBUF."""
    return ap.bitcast(FP32R)


def build_nc(causal: bool):
    nc = bacc.Bacc(trn_type="TRN2", target_bir_lowering=False, debug=False)

    xT_d = nc.declare_dram_parameter("xT", [D, S], FP32, isOutput=False)
    wqT_d = nc.declare_dram_parameter("wqT", [D, HD], FP32, isOutput=False)
    wkT_d = nc.declare_dram_parameter("wkT", [D, HD], FP32, isOutput=False)
    wvT_d = nc.declare_dram_parameter("wvT", [D, VP], FP32, isOutput=False)
    woT_d = nc.declare_dram_parameter("woT", [HD, D], FP32, isOutput=False)
    bq_d = nc.declare_dram_parameter("bq", [HD], FP32, isOutput=False)
    bk_d = nc.declare_dram_parameter("bk", [HD], FP32, isOutput=False)
    bv_d = nc.declare_dram_parameter("bv", [VP], FP32, isOutput=False)
    cm_d = nc.declare_dram_parameter("cmask", [128, 2 * SB], FP32, isOutput=False)
    yT_d = nc.declare_dram_parameter("yT", [D, S], FP32, isOutput=True)

    EXP = mybir.ActivationFunctionType.Exp

    with tile.TileContext(nc) as tc, ExitStack() as ctx:
        consts = ctx.enter_context(tc.tile_pool(name="consts", bufs=1))

        # ---- constant / persistent SBUF tensors ----
        xT_sb = consts.tile([128, KC, S], FP32R)  # x^T, chunk c = rows 128c..
        wqT_sb = consts.tile([128, KC, HD], FP32R)
        wkT_sb = consts.tile([128, KC, HD], FP32R)
        wvT_sb = consts.tile([128, KC, VP], FP32R)
        woT_sb = [consts.tile([64, D], FP32R, name=f"woT{h}") for h in range(HG)]
        bq0_sb = consts.tile([128, 1], FP32, name="bq0")
        bq1_sb = consts.tile([64, 1], FP32, name="bq1")
        bk0_sb = consts.tile([128, 1], FP32, name="bk0")
        bk1_sb = consts.tile([64, 1], FP32, name="bk1")
        bvb_sb = consts.tile([128, VP], FP32)  # bv broadcast to all partitions
        v65_sb = consts.tile([128, NT, HG, 65], FP32R)  # v tiles + ones column
        qT01_sb = consts.tile([128, S], FP32R)  # q^T heads 0,1
        qT2_sb = consts.tile([64, S], FP32R)  # q^T head 2
        kT01_sb = consts.tile([128, S], FP32R)
        kT2_sb = consts.tile([64, S], FP32R)
        if causal:
            cm_sb = consts.tile([128, 2 * SB], FP32)
            nc.sync.dma_start(out=cm_sb, in_=cm_d.ap())

        # v-projection weights first, then x column-block by column-block so
        # the v projection can start after ~1.5MB instead of the full 8.7MB.
        wvT_r = wvT_d.ap().rearrange("(c p) n -> p c n", p=128)
        nc.sync.dma_start(out=wvT_sb[:, 0 : KC // 2, :], in_=_r(wvT_r[:, 0 : KC // 2, :]))
        nc.gpsimd.dma_start(out=wvT_sb[:, KC // 2 :, :], in_=_r(wvT_r[:, KC // 2 :, :]))
        xT_r = xT_d.ap().rearrange("(c p) s -> p c s", p=128)

        def load_x_block(jb, split=False):
            for c in range(KC):
                eng = nc.gpsimd if split and c >= KC // 2 else nc.sync
                eng.dma_start(
                    out=xT_sb[:, c, jb * SB : (jb + 1) * SB],
                    in_=_r(xT_r[:, c, jb * SB : (jb + 1) * SB]),
                )

        # tiny constants first so nothing downstream waits on them
        nc.sync.dma_start(
            out=bq0_sb, in_=bq_d.ap()[0:128].rearrange("(p o) -> p o", o=1)
        )
        nc.sync.dma_start(
            out=bq1_sb, in_=bq_d.ap()[128:192].rearrange("(p o) -> p o", o=1)
        )
        nc.sync.dma_start(
            out=bk0_sb, in_=bk_d.ap()[0:128].rearrange("(p o) -> p o", o=1)
        )
        nc.sync.dma_start(
            out=bk1_sb, in_=bk_d.ap()[128:192].rearrange("(p o) -> p o", o=1)
        )
        # broadcast bv across all 128 partitions with a step-0 partition AP
        bv_ap = bv_d.ap()
        bvb_src = bass.AP(
            tensor=bv_ap.tensor, offset=bv_ap.offset, ap=[[0, 128], [1, VP]]
        )
        nc.sync.dma_start(out=bvb_sb, in_=bvb_src)

        # x block 0 on the Pool queue, q/k weights on SP — both land ~6us in
        # so the first attention block starts early. Outputs + rz hops also
        # use the Pool queue so they don't wait behind bulk input loads.
        for c in range(KC):
            nc.gpsimd.dma_start(out=xT_sb[:, c, 0:SB], in_=_r(xT_r[:, c, 0:SB]))
        nc.sync.dma_start(
            out=wqT_sb, in_=_r(wqT_d.ap().rearrange("(c p) n -> p c n", p=128))
        )
        nc.sync.dma_start(
            out=wkT_sb, in_=_r(wkT_d.ap().rearrange("(c p) n -> p c n", p=128))
        )
        load_x_block(1)
        for h in range(HG):
            nc.gpsimd.dma_start(
                out=woT_sb[h], in_=_r(woT_d.ap()[h * 64 : (h + 1) * 64, :])
            )
        load_x_block(2)
        load_x_block(3)

        nc.vector.memset(v65_sb.bitcast(FP32), 1.0)  # preset ones column

        # One fused per-block pipeline: for each 512-column sequence block,
        # project v/q/k for that block, then run attention + output
        # projection. Each block only depends on x columns loaded so far, so
        # compute streams behind the DMA.
        proj_pool = ctx.enter_context(
            tc.tile_pool(name="proj", bufs=PSUM_BUFS[0], space="PSUM")
        )
        sp_pool = ctx.enter_context(
            tc.tile_pool(name="sp", bufs=PSUM_BUFS[1], space="PSUM")
        )
        op_pool = ctx.enter_context(
            tc.tile_pool(name="op", bufs=PSUM_BUFS[2], space="PSUM")
        )
        yp_pool = ctx.enter_context(
            tc.tile_pool(name="yp", bufs=PSUM_BUFS[3], space="PSUM")
        )
        et_pool = ctx.enter_context(tc.tile_pool(name="et", bufs=3))
        ef_pool = ctx.enter_context(tc.tile_pool(name="ef", bufs=2))
        rc_pool = ctx.enter_context(tc.tile_pool(name="rc", bufs=2))
        ot_pool = ctx.enter_context(tc.tile_pool(name="ot", bufs=6))
        yt_pool = ctx.enter_context(tc.tile_pool(name="yt", bufs=4))

        def project_block(j):
            # v projection for this block's 4 key tiles (x^T stationary)
            for st in range(4 * j, 4 * (j + 1)):
                vp = proj_pool.tile([128, VP], FP32, name="vp", tag="proj")
                for c in range(KC):
                    nc.tensor.matmul(
                        vp,
                        lhsT=xT_sb[:, c, st * 128 : (st + 1) * 128],
                        rhs=wvT_sb[:, c, :],
                        start=(c == 0),
                        stop=(c == KC - 1),
                    )
                for h in range(HG):
                    nc.vector.tensor_add(
                        v65_sb[:, st, h, 0:64],
                        vp[:, h * 64 : (h + 1) * 64],
                        bvb_sb[:, h * 64 : (h + 1) * 64],
                    )

            # q/k projections for this block (w stationary, transposed out)
            for w_sb, b0, b1, dst01, dst2 in (
                (wqT_sb, bq0_sb, bq1_sb, qT01_sb, qT2_sb),
                (wkT_sb, bk0_sb, bk1_sb, kT01_sb, kT2_sb),
            ):
                for mt, m, dst, bias in ((0, 128, dst01, b0), (1, 64, dst2, b1)):
                    pp = proj_pool.tile([128, SB], FP32, name="pp", tag="proj")
                    for c in range(KC):
                        nc.tensor.matmul(
                            pp[0:m, :],
                            lhsT=w_sb[:, c, mt * 128 : mt * 128 + m],
                            rhs=xT_sb[:, c, j * SB : (j + 1) * SB],
                            start=(c == 0),
                            stop=(c == KC - 1),
                        )
                    nc.vector.tensor_scalar_add(
                        dst[0:m, j * SB : (j + 1) * SB], pp[0:m, :], bias[0:m, :]
                    )

        def attend_block(j):
            out_tiles = []
            for h in range(HG):
                if h < 2:
                    qsrc, ksrc, base = qT01_sb, kT01_sb, 64 * h
                else:
                    qsrc, ksrc, base = qT2_sb, kT2_sb, 0
                tend = 4 * (j + 1) if causal else NT
                ndiag = tend - 4 * j if causal else 0  # trailing diagonal tiles
                nfull = tend - ndiag
                op = op_pool.tile([65, SB], FP32)

                def scores(dst, t, off=0):
                    nc.tensor.matmul(
                        dst,
                        lhsT=ksrc[base : base + 64, t * 128 : (t + 1) * 128],
                        rhs=qsrc[base : base + 64, j * SB + off : (j + 1) * SB],
                        start=True,
                        stop=True,
                    )

                def attnv(t, et_ap, off=0):
                    nc.tensor.matmul(
                        op[:, off:SB],
                        lhsT=v65_sb[:, t, h, :],
                        rhs=et_ap,
                        start=(t == 0),
                        stop=(t == tend - 1),
                    )

                # full (off-diagonal) tiles
                for t in range(nfull):
                    sp = sp_pool.tile([128, SB], FP32)
                    scores(sp, t)
                    et = et_pool.tile([128, SB], FP32R)
                    nc.scalar.activation(et, sp, EXP, scale=SCALE)
                    attnv(t, et)
                # diagonal tiles: trim to useful causal width (but keep
                # N>=256 so float32r stays at 1 cycle/row; the extra masked
                # columns are zeroed by the mask), exp, then multiply by the
                # 0/1 mask (keep iff p <= c_local - d)
                for t in range(nfull, tend):
                    off = min(128 * t - SB * j, SB - 256)
                    d = 128 * t - SB * j - off
                    n = SB - off
                    sp = sp_pool.tile([128, SB], FP32)
                    scores(sp[:, 0:n], t, off)
                    et = et_pool.tile([128, SB], FP32R)
                    ef = ef_pool.tile([128, SB], FP32)
                    nc.scalar.activation(ef[:, 0:n], sp[:, 0:n], EXP, scale=SCALE)
                    nc.vector.tensor_mul(
                        et[:, 0:n], ef[:, 0:n], cm_sb[:, SB - d : 2 * SB - d - off]
                    )
                    attnv(t, et[:, 0:n], off)
                # normalize: rows 0:64 / row 64 (gpsimd partition broadcast).
                # partition_broadcast HW ucode reads partition 0 regardless of
                # the AP offset, so DMA-hop the reciprocal row to partition 0.
                rc = rc_pool.tile([65, SB], FP32)
                nc.vector.reciprocal(rc[64:65, :], op[64:65, :])
                rz = rc_pool.tile([1, SB], FP32, name="rz")
                nc.gpsimd.dma_start(out=rz, in_=rc[64:65, :])
                bc = rc_pool.tile([64, SB], FP32, name="bc")
                nc.gpsimd.partition_broadcast(bc, rz[0:1, :])
                ot = ot_pool.tile([64, SB], FP32R)
                nc.vector.tensor_mul(ot, op[0:64, :], bc)
                out_tiles.append(ot)

            for dt in range(KC):
                yp = yp_pool.tile([128, SB], FP32, name="yp")
                for h in range(HG):
                    nc.tensor.matmul(
                        yp,
                        lhsT=woT_sb[h][:, dt * 128 : (dt + 1) * 128],
                        rhs=out_tiles[h],
                        start=(h == 0),
                        stop=(h == HG - 1),
                    )
                yt = yt_pool.tile([128, SB], FP32)
                nc.vector.tensor_copy(yt, yp)
                # y outputs ride the SP queue: each block's outputs trail the
                # input stream, so they never contend with it, and keeping
                # them off the Pool queue unclogs the rz/broadcast hops
                nc.sync.dma_start(
                    out=yT_d.ap()[dt * 128 : (dt + 1) * 128, j * SB : (j + 1) * SB],
                    in_=yt,
                )

        if causal:
            # fused: attention j only needs k/v tiles t < 4(j+1)
            for j in range(NJ):
                project_block(j)
                attend_block(j)
        else:
            # full attention needs all k/v before any attention block
            for j in range(NJ):
                project_block(j)
            for j in range(NJ):
                attend_block(j)

    nc.finalize()
    return nc


_NC_CACHE: dict[bool, object] = {}


def get_nc(causal: bool):
    if causal not in _NC_CACHE:
        _NC_CACHE[causal] = build_nc(causal)
    return _NC_CACHE[causal]


def _make_cmask():
    # cmask[p, u] = 1.0 iff p <= u - SB   (slice at s0 = SB + SB*j - 128*t
    # gives keep iff 128t+p <= 512j+c)
    p = np.arange(128)[:, None]
    u = np.arange(2 * SB)[None, :]
    return (p <= u - SB).astype(np.float32)


def make_in_maps(x, wq, bq, wk, bk, wv, bv, wo, bo):
    """Shard full inputs into 8 per-core input maps."""
    f32 = np.float32
    cmask = _make_cmask()
    in_maps = []
    for core in range(NCORES):
        b, hg = divmod(core, NH // HG)
        hs = slice(hg * HD, (hg + 1) * HD)
        wvT = np.zeros((D, VP), f32)
        wvT[:, :HD] = wv[hs, :].T
        bvp = np.zeros((VP,), f32)
        bvp[:HD] = bv[hs]
        in_maps.append(
            {
                "xT": np.ascontiguousarray(x[b].T, f32),
                "wqT": np.ascontiguousarray(wq[hs, :].T, f32),
                "wkT": np.ascontiguousarray(wk[hs, :].T, f32),
                "wvT": wvT,
                "woT": np.ascontiguousarray(wo[:, hs].T, f32),
                "bq": np.ascontiguousarray(bq[hs], f32),
                "bk": np.ascontiguousarray(bk[hs], f32),
                "bv": bvp,
                "cmask": cmask,
            }
        )
    return in_maps


def combine_outputs(results, bo):
    """Sum head-group partials per batch, transpose, add output bias."""
    y = np.empty((B, S, D), np.float32)
    ng = NH // HG
    for b in range(B):
        acc = results[b * ng]["yT"].astype(np.float32)
        for g in range(1, ng):
            acc = acc + results[b * ng + g]["yT"]
        y[b] = acc.T + np.asarray(bo, np.float32)[None, :]
    return y


def kernel(x, wq, bq, wk, bk, wv, bv, wo, bo, mask, _trace=False):
    from concourse.bass_utils import run_bass_kernel_spmd

    causal = bool(np.asarray(mask).item())
    nc = get_nc(causal)
    in_maps = make_in_maps(x, wq, bq, wk, bk, wv, bv, wo, bo)
    res = run_bass_kernel_spmd(nc, in_maps, list(range(NCORES)), trace=_trace)
    y = combine_outputs(res.results, bo)
    if _trace:
        return y, res
    return y



# revision 3
# speedup vs baseline: 1.0039x; 1.0039x over previous
"""Trainium2 Bass kernel for 12-head causal MHA (B=2, S=2048, D=768), bf16 compute.

Sharding: 8 cores = (batch b in {0,1}) x (head-group hg in {0..3}, 3 heads each).

Per-core structure (per 512-column sequence block j):
  - v projection (x-stationary, natural [keys, vdim] layout, N=192)
  - q/k projections packed into 3 groups of 128 output rows:
      g0 = q heads 0,1 | g1 = q head 2 + k head 0 | g2 = k heads 1,2
    (k bias is mathematically irrelevant under softmax and is skipped;
     q bias applied via per-partition tensor_scalar on the PSUM->SBUF copy)
  - scores [keys, q] per 128-key tile; full tiles exp'd in [128,1024] pairs,
    diagonal tiles exp'd at exact causal width then masked (constant 0/1
    mask cm4, same pattern for every block)
  - attn@V transposed: out[q, 65] = et^T @ v65 (65 = 64 vdims + ones col for
    the softmax denominator) -- free size 65 instead of 512 halves PE cost
  - normalize per-partition (query) via reciprocal of col 64 + broadcast mul
  - DMA-transpose (XBAR) ot [q, hd] -> otT [hd, q] SBUF->SBUF, heads 0,1
    packed on partitions 0..127 so the output projection contracts 192 dims
    in 2 K-groups
  - y^T partial = wo^T @ otT accumulated over 2 K-groups, copied to bf16 on
    gpsimd, DMA'd out

Host sums the 4 head-group partials per batch (fp32), transposes, and adds
bo + wo @ bv (bv is folded out of the device kernel).
"""

import math
from contextlib import ExitStack

import numpy as np

import concourse.bacc as bacc
import concourse.bass as bass
import concourse.mybir as mybir
import concourse.tile as tile

FP32 = mybir.dt.float32
BF16 = mybir.dt.bfloat16

B = 2
S = 2048
D = 768
NH = 12
DK = 64
NCORES = 8
HG = 3  # heads per core
HD = HG * DK  # 192
KC = D // 128  # 6 contraction chunks
SB = 512  # sequence block
NJ = S // SB  # 4
NT = S // 128  # 16 key tiles
SCALE = 1.0 / math.sqrt(DK)
EXP = mybir.ActivationFunctionType.Exp


def build_nc(causal: bool):
    nc = bacc.Bacc(trn_type="TRN2", target_bir_lowering=False, debug=False)

    x6_d = nc.declare_dram_parameter("x6", [128, KC, S], BF16, isOutput=False)
    wqk_d = nc.declare_dram_parameter("wqk", [128, KC, 3 * 128], BF16, isOutput=False)
    # wqk groups: g0 = wq heads 0,1 | g1 = wk heads 0,1 | g2 = [wq h2 | wk h2]
    wv6_d = nc.declare_dram_parameter("wv6", [128, KC, HD], BF16, isOutput=False)
    wo0_d = nc.declare_dram_parameter("wo0", [128, D], BF16, isOutput=False)
    wo1_d = nc.declare_dram_parameter("wo1", [64, D], BF16, isOutput=False)
    bqg_d = nc.declare_dram_parameter("bqg", [128, 2], FP32, isOutput=False)
    cm4_d = nc.declare_dram_parameter("cm4", [128, SB], BF16, isOutput=False)
    yT_d = nc.declare_dram_parameter("yT", [D, S], BF16, isOutput=True)

    with tile.TileContext(nc) as tc, ExitStack() as ctx:
        consts = ctx.enter_context(tc.tile_pool(name="consts", bufs=1))

        x6_sb = consts.tile([128, KC, S], BF16)
        wqk_sb = consts.tile([128, KC, 3 * 128], BF16)
        wv6_sb = consts.tile([128, KC, HD], BF16)
        wo0_sb = consts.tile([128, D], BF16)
        wo1_sb = consts.tile([64, D], BF16)
        bqg_sb = consts.tile([128, 2], FP32)
        cm4_sb = consts.tile([128, SB], BF16)
        qT01_sb = consts.tile([128, S], BF16)  # q heads 0,1
        kT01_sb = consts.tile([128, S], BF16)  # k heads 0,1
        qT2_sb = consts.tile([64, S], BF16)  # q head 2
        kT2_sb = consts.tile([64, S], BF16)  # k head 2
        v65_sb = consts.tile([128, NT, HG, 65], BF16)

        # ---- input DMAs: v weights (SWDGE path, parallel with HWDGE) + x
        # tile 0 first so compute starts early
        nc.gpsimd.dma_start(out=wv6_sb, in_=wv6_d.ap())
        nc.sync.dma_start(out=x6_sb[:, :, 0:128], in_=x6_d.ap()[:, :, 0:128])
        nc.sync.dma_start(out=x6_sb[:, :, 128:SB], in_=x6_d.ap()[:, :, 128:SB])
        nc.sync.dma_start(out=wqk_sb[:, :, 0:128], in_=wqk_d.ap()[:, :, 0:128])
        nc.sync.dma_start(out=wqk_sb[:, :, 128:256], in_=wqk_d.ap()[:, :, 128:256])
        nc.sync.dma_start(out=wqk_sb[:, :, 256:384], in_=wqk_d.ap()[:, :, 256:384])
        nc.scalar.dma_start(out=bqg_sb, in_=bqg_d.ap())
        if causal:
            nc.scalar.dma_start(out=cm4_sb, in_=cm4_d.ap())
        nc.sync.dma_start(
            out=x6_sb[:, :, SB : 2 * SB], in_=x6_d.ap()[:, :, SB : 2 * SB]
        )
        nc.scalar.dma_start(out=wo0_sb, in_=wo0_d.ap())
        nc.scalar.dma_start(out=wo1_sb, in_=wo1_d.ap())
        for j in range(2, NJ):
            eng = nc.sync if j < 3 else nc.scalar
            eng.dma_start(
                out=x6_sb[:, :, j * SB : (j + 1) * SB],
                in_=x6_d.ap()[:, :, j * SB : (j + 1) * SB],
            )

        # ones column for the softmax denominator
        nc.vector.memset(v65_sb[:, :, :, 64:65], 1.0)

        sp_pool = ctx.enter_context(tc.tile_pool(name="sp", bufs=2, space="PSUM"))
        pj_pool = ctx.enter_context(tc.tile_pool(name="pj", bufs=2, space="PSUM"))
        oq_pool = ctx.enter_context(tc.tile_pool(name="oq", bufs=2, space="PSUM"))
        et_pool = ctx.enter_context(tc.tile_pool(name="et", bufs=4))
        ot_pool = ctx.enter_context(tc.tile_pool(name="ot", bufs=2))
        rc_pool = ctx.enter_context(tc.tile_pool(name="rc", bufs=2))
        oT_pool = ctx.enter_context(tc.tile_pool(name="oT", bufs=2))
        yt_pool = ctx.enter_context(tc.tile_pool(name="yt", bufs=3))

        def q_ap(h, j):  # [64, SB] moving operand for scores
            src, base = (
                (qT01_sb, 0) if h == 0 else (qT01_sb, 64) if h == 1 else (qT2_sb, 0)
            )
            return src[base : base + 64, j * SB : (j + 1) * SB]

        def k_ap(h, t):  # [64, 128] stationary operand for scores
            src, base = (
                (kT01_sb, 0) if h == 0 else (kT01_sb, 64) if h == 1 else (kT2_sb, 0)
            )
            return src[base : base + 64, t * 128 : (t + 1) * 128]

        def projv_units(j):
            # v projection: x-stationary, per key tile, N=192
            for st in range(4 * j, 4 * (j + 1)):
                def unit(st=st):
                    vp = pj_pool.tile([128, SB], FP32, name="pj")
                    for c in range(KC):
                        nc.tensor.matmul(
                            vp[:, 0:HD],
                            lhsT=x6_sb[:, c, st * 128 : (st + 1) * 128],
                            rhs=wv6_sb[:, c, :],
                            start=(c == 0),
                            stop=(c == KC - 1),
                        )
                    nc.vector.tensor_copy(
                        v65_sb[:, st, :, 0:64],
                        vp[:, 0:HD].rearrange("p (h d) -> p h d", h=HG),
                    )
                yield unit

        def projqk_units(j, part=None):
            # q/k projections, w-stationary: two 128-row groups (q01, k01)
            # and two 64-row groups (q2, k2) so scores operands share a
            # partition base per head. part "a" = heads 0,1; "b" = head 2.
            jsp = slice(j * SB, (j + 1) * SB)
            groups = (
                (0, 128, 0, qT01_sb, bqg_sb[:, 0:1]),
                (1, 128, 0, kT01_sb, None),
                (2, 64, 0, qT2_sb, bqg_sb[0:64, 1:2]),
                (2, 64, 64, kT2_sb, None),
            )
            if part == "a":
                groups = groups[0:2]
            elif part == "b":
                groups = groups[2:4]
            for g, m, w0, dst, bias in groups:
                def unit(g=g, m=m, w0=w0, dst=dst, bias=bias):
                    pp = pj_pool.tile([128, SB], FP32, name="pj")
                    for c in range(KC):
                        nc.tensor.matmul(
                            pp[0:m, :],
                            lhsT=wqk_sb[:, c, g * 128 + w0 : g * 128 + w0 + m],
                            rhs=x6_sb[:, c, jsp],
                            start=(c == 0),
                            stop=(c == KC - 1),
                        )
                    if bias is not None:
                        nc.vector.tensor_scalar_add(dst[:, jsp], pp[0:m, :], bias)
                    else:
                        nc.vector.tensor_copy(dst[:, jsp], pp[0:m, :])
                yield unit

        def scores_full_units(j, h, et_t):
            """Full (off-diagonal) score tiles of one head, exp'd in pairs."""
            nfull = 4 * j if causal else NT
            for t0 in range(0, nfull, 2):
                def full_pair(t0=t0):
                    spf = sp_pool.tile([128, 2 * SB], FP32, name="sp")
                    for u in range(2):
                        nc.tensor.matmul(
                            spf[:, u * SB : (u + 1) * SB],
                            lhsT=k_ap(h, t0 + u),
                            rhs=q_ap(h, j),
                            start=True,
                            stop=True,
                        )
                    nc.scalar.activation(
                        et_t[:, t0 : t0 + 2, :], spf, EXP, scale=SCALE
                    )
                yield full_pair

        def scores_diag_units(j, h, et_t):
            """Diagonal score tiles at exact causal width, then 0/1 mask."""
            for u in range(4):
                def diag(u=u):
                    t = 4 * j + u
                    off = 128 * u
                    n = SB - off
                    spd = sp_pool.tile([128, 2 * SB], FP32, name="sp")
                    nc.tensor.matmul(
                        spd[:, 0:n],
                        lhsT=k_ap(h, t),
                        rhs=q_ap(h, j)[:, off:SB],
                        start=True,
                        stop=True,
                    )
                    nc.scalar.activation(
                        et_t[:, t, off:SB], spd[:, 0:n], EXP, scale=SCALE
                    )
                    nc.vector.tensor_mul(
                        et_t[:, t, off:SB],
                        et_t[:, t, off:SB],
                        cm4_sb[:, 0:n],
                    )
                yield diag

        def attnv_units(j, h, et_t, rc_t, ot4):
            """attn@V chains + normalize for one head."""
            tend = 4 * (j + 1) if causal else NT
            oq_t = oq_pool.tile([128, 4, 128], FP32)
            for qs in range(4):
                def chain(qs=qs):
                    tq = (4 * j + qs + 1) if causal else tend
                    for t in range(tq):
                        nc.tensor.matmul(
                            oq_t[:, qs, 0:65],
                            lhsT=et_t[:, t, qs * 128 : (qs + 1) * 128],
                            rhs=v65_sb[:, t, h, :],
                            start=(t == 0),
                            stop=(t == tq - 1),
                        )
                yield chain

            def normalize():
                # normalize immediately so the oq buffer frees early:
                # ot4[q, qs, h, :] = oq[q, qs, 0:64] / oq[q, qs, 64]
                nc.vector.reciprocal(rc_t[:, h * 4 : (h + 1) * 4], oq_t[:, :, 64])
                nc.vector.tensor_mul(
                    ot4[:, :, h, :],
                    oq_t[:, :, 0:64],
                    rc_t[:, h * 4 : (h + 1) * 4].unsqueeze(-1).to_broadcast(
                        (128, 4, 64)
                    ),
                )
            yield normalize

        def yproj_units(j, oT01, oT2, copy_engines):
            jsp = slice(j * SB, (j + 1) * SB)
            # output projection: 2 contraction groups (128 + 64)
            for dt in range(KC):
                def unit(dt=dt):
                    yp = pj_pool.tile([128, SB], FP32, name="pj")
                    nc.tensor.matmul(
                        yp,
                        lhsT=wo0_sb[:, dt * 128 : (dt + 1) * 128],
                        rhs=oT01,
                        start=True,
                        stop=False,
                    )
                    nc.tensor.matmul(
                        yp,
                        lhsT=wo1_sb[:, dt * 128 : (dt + 1) * 128],
                        rhs=oT2[0:64, :],
                        start=False,
                        stop=True,
                    )
                    yt = yt_pool.tile([128, SB], BF16)
                    eng = copy_engines[dt % len(copy_engines)]
                    if eng is nc.scalar:
                        eng.copy(yt, yp)
                    else:
                        eng.tensor_copy(yt, yp)
                    nc.sync.dma_start(
                        out=yT_d.ap()[dt * 128 : (dt + 1) * 128, jsp], in_=yt
                    )
                yield unit

        def interleave(feeder, filler):
            """Emit feeder units (which keep Act busy) with filler PE units
            spread evenly between them; leftover fillers go at the end."""
            feeder = list(feeder)
            filler = list(filler)
            nf = len(feeder)
            emitted = 0
            for i, f in enumerate(feeder):
                f()
                want = (i + 1) * len(filler) // nf if nf else len(filler)
                while emitted < want:
                    filler[emitted]()
                    emitted += 1
            while emitted < len(filler):
                filler[emitted]()
                emitted += 1

        def transposes01(ot4):
            # XBAR transposes for heads 0,1 (packed on partitions 0..127)
            oT01 = oT_pool.tile([128, SB], BF16, name="oT01")
            for qs in range(4):
                nc.sync.dma_start_transpose(
                    out=oT01[:, qs * 128 : (qs + 1) * 128], in_=ot4[:, qs, 0:2, :]
                )
            return oT01

        def transposes2(ot4):
            # XBAR transpose for head 2 (+pad rows, never consumed)
            oT2 = oT_pool.tile([128, SB], BF16, name="oT2")
            for qs in range(4):
                nc.scalar.dma_start_transpose(
                    out=oT2[:, qs * 128 : (qs + 1) * 128], in_=ot4[:, qs, 2:4, :]
                )
            return oT2

        def run(units):
            for u in units:
                u()

        # Software-pipelined global schedule driven by virtual PE/Act
        # clocks: score+exp units are "feeders" (they load both engines),
        # everything else is PE-only "filler". A feeder is emitted when the
        # Act backlog is small (sp pool depth limits PE run-ahead anyway);
        # fillers drain while Act chews. Fillers carry across phases.
        PE_CYC = 0.4167

        def fp_cost(_):  # full pair: 2 scores + [128,1024] exp
            return 2 * SB * PE_CYC, 1024 * 0.833 + 185

        def dg_cost(u):  # diag tile u: score + exp + mask
            n = SB - 128 * u
            return n * PE_CYC, n * 0.833 + 185

        class tposes:
            oT01 = None
            oT2 = None

        if causal:
            clocks = {"pe": 0.0, "act": 0.0}
            fillers = []

            def emit_feeder(u, pe, act):
                u()
                clocks["pe"] += pe
                clocks["act"] = max(clocks["act"], clocks["pe"]) + act

            def emit_filler():
                pe, u = fillers.pop(0)
                u()
                clocks["pe"] += pe

            def phase(feeders):
                for u, pe, act in feeders:
                    # drain fillers while Act has >1.4us of backlog
                    while fillers and clocks["act"] - clocks["pe"] > 1400:
                        emit_filler()
                    emit_feeder(u, pe, act)

            def add_fillers(units, pe_each):
                fillers.extend((pe_each, u) for u in units)

            pv0 = list(projv_units(0))
            run(pv0)
            for u in projqk_units(0, "a"):
                u()
                clocks["pe"] += 1280
            ets = {}

            def et(j, h):
                if (j, h) not in ets:
                    ets[(j, h)] = et_pool.tile([128, NT, SB], BF16, name="et")
                return ets[(j, h)]

            def feed_full(j, h):
                return [(u, *fp_cost(0)) for u in scores_full_units(j, h, et(j, h))]

            def feed_diag(j, h):
                return [
                    (u, *dg_cost(i))
                    for i, u in enumerate(scores_diag_units(j, h, et(j, h)))
                ]

            pqkb_done = {}

            def mark(j):
                def m():
                    pqkb_done[j] = True
                return m

            for j in range(NJ):
                rc_t = rc_pool.tile([128, HG * 4], FP32)
                ot4 = ot_pool.tile([128, 4, 4, 64], BF16)
                nc.vector.memset(ot4[:, :, 3, :], 0.0)
                if j == 0:
                    add_fillers(projqk_units(0, "b"), 1280)
                    add_fillers([mark(0)], 0)
                    phase(feed_full(0, 0) + feed_diag(0, 0))
                else:
                    phase(feed_diag(j, 0))
                av0 = list(attnv_units(j, 0, ets.pop((j, 0)), rc_t, ot4))
                add_fillers(av0, 27 * (4 * j + 3))
                phase(feed_full(j, 1) + feed_diag(j, 1))
                av1 = list(attnv_units(j, 1, ets.pop((j, 1)), rc_t, ot4))
                add_fillers(av1, 27 * (4 * j + 3))
                add_fillers(
                    [lambda ot4=ot4: setattr(tposes, "oT01", transposes01(ot4))], 0
                )
                # head-2 q/k of this block must be in SBUF before its scores
                while fillers and not pqkb_done.get(j, False):
                    emit_filler()
                phase(feed_full(j, 2) + feed_diag(j, 2))
                if j + 1 < NJ:
                    av2 = list(attnv_units(j, 2, ets.pop((j, 2)), rc_t, ot4))
                    # q01/k01 of the next block precede its scores; Act still
                    # has the h2-scores backlog to chew while PE projects
                    while fillers:
                        emit_filler()
                    for u in projqk_units(j + 1, "a"):
                        u()
                        clocks["pe"] += 1280
                    add_fillers(av2, 27 * (4 * j + 3))
                    add_fillers(
                        [lambda ot4=ot4: setattr(tposes, "oT2", transposes2(ot4))], 0
                    )
                    phase(feed_full(j + 1, 0))
                    add_fillers(projqk_units(j + 1, "b"), 1280)
                    add_fillers([mark(j + 1)], 0)
                    add_fillers(projv_units(j + 1), 480)
                    while fillers and tposes.oT2 is None:
                        emit_filler()
                    oT01, oT2 = tposes.oT01, tposes.oT2
                    tposes.oT01 = tposes.oT2 = None
                    add_fillers(
                        yproj_units(j, oT01, oT2, (nc.vector, nc.vector, nc.vector)),
                        426,
                    )
                else:
                    # ---- last block: process head 2 per query-subtile so
                    # the XBAR transposes and the output projection pipeline
                    # with the remaining attn@V chains
                    et_t = ets.pop((j, 2))
                    oq_t = oq_pool.tile([128, 4, 128], FP32)
                    oT2 = oT_pool.tile([128, SB], BF16, name="oT2")
                    for qs in range(4):
                        tq = 4 * j + qs + 1
                        for t in range(tq):
                            nc.tensor.matmul(
                                oq_t[:, qs, 0:65],
                                lhsT=et_t[:, t, qs * 128 : (qs + 1) * 128],
                                rhs=v65_sb[:, t, 2, :],
                                start=(t == 0),
                                stop=(t == tq - 1),
                            )
                        nc.vector.reciprocal(
                            rc_t[:, 8 + qs : 9 + qs], oq_t[:, qs, 64:65]
                        )
                        nc.vector.tensor_mul(
                            ot4[:, qs, 2, :],
                            oq_t[:, qs, 0:64],
                            rc_t[:, 8 + qs : 9 + qs].to_broadcast((128, 64)),
                        )
                        eng = nc.scalar if qs % 2 else nc.sync
                        eng.dma_start_transpose(
                            out=oT2[:, qs * 128 : (qs + 1) * 128],
                            in_=ot4[:, qs, 2:4, :],
                        )
                        if qs == 1:
                            while fillers:
                                emit_filler()
                    oT01 = tposes.oT01
                    tposes.oT01 = None
                    run(yproj_units(j, oT01, oT2, (nc.vector, nc.scalar, nc.vector)))
            while fillers:
                emit_filler()
        else:
            run(projqk_units(0))
            for j in range(NJ):
                run(projv_units(j))
                if j + 1 < NJ:
                    run(projqk_units(j + 1))
            yprev = []
            for j in range(NJ):
                rc_t = rc_pool.tile([128, HG * 4], FP32)
                ot4 = ot_pool.tile([128, 4, 4, 64], BF16)
                nc.vector.memset(ot4[:, :, 3, :], 0.0)
                ets = [
                    et_pool.tile([128, NT, SB], BF16, name="et") for _ in range(HG)
                ]
                interleave(scores_full_units(j, 0, ets[0]), yprev)
                interleave(
                    scores_full_units(j, 1, ets[1]),
                    attnv_units(j, 0, ets[0], rc_t, ot4),
                )
                interleave(
                    scores_full_units(j, 2, ets[2]),
                    attnv_units(j, 1, ets[1], rc_t, ot4),
                )
                run(attnv_units(j, 2, ets[2], rc_t, ot4))
                oT01 = transposes01(ot4)
                oT2 = transposes2(ot4)
                engines = (nc.vector, nc.vector, nc.vector)
                yprev = list(yproj_units(j, oT01, oT2, engines))
                for u in yprev:
                    u()
                yprev = []

    nc.finalize()
    return nc


_NC_CACHE: dict[bool, object] = {}


def get_nc(causal: bool):
    if causal not in _NC_CACHE:
        _NC_CACHE[causal] = build_nc(causal)
    return _NC_CACHE[causal]


def _bf16(a):
    import ml_dtypes

    return np.asarray(a, np.float32).astype(ml_dtypes.bfloat16)


def _chunked(wT):
    """[768, N] -> [128, 6, N] with chunk c = rows 128c..128c+127."""
    n = wT.shape[1]
    return np.ascontiguousarray(wT.reshape(KC, 128, n).transpose(1, 0, 2))


def _make_cm4():
    # relative triangle mask: cm4[p, c] = 1.0 iff c >= p (every diagonal
    # tile uses the width-n prefix of this pattern)
    p = np.arange(128)[:, None]
    c = np.arange(SB)[None, :]
    return (c >= p).astype(np.float32)


def make_in_maps(x, wq, bq, wk, bk, wv, bv, wo, bo):
    f32 = np.float32
    x = np.asarray(x, f32)
    cm4 = _bf16(_make_cm4())
    in_maps = []
    for core in range(NCORES):
        b, hg = divmod(core, NH // HG)
        hs = slice(hg * HD, (hg + 1) * HD)
        wqT = np.asarray(wq, f32)[hs, :].T  # [768, 192]
        wkT = np.asarray(wk, f32)[hs, :].T
        wqkT = np.concatenate(
            [wqT[:, 0:128], wkT[:, 0:128], wqT[:, 128:192], wkT[:, 128:192]], axis=1
        )
        bqc = np.asarray(bq, f32)[hs]
        bqg = np.zeros((128, 2), f32)
        bqg[:, 0] = bqc[0:128]
        bqg[0:64, 1] = bqc[128:192]
        woT = np.asarray(wo, f32)[:, hs].T  # [192, 768]
        in_maps.append(
            {
                "x6": _bf16(_chunked(np.ascontiguousarray(x[b].T))),
                "wqk": _bf16(_chunked(wqkT)),
                "wv6": _bf16(_chunked(np.asarray(wv, f32)[hs, :].T)),
                "wo0": _bf16(woT[0:128, :]),
                "wo1": _bf16(woT[128:192, :]),
                "bqg": bqg,
                "cm4": cm4,
            }
        )
    return in_maps


def combine_outputs(results, wo, bv, bo):
    y = np.empty((B, S, D), np.float32)
    ng = NH // HG
    extra = (np.asarray(wo, np.float32) @ np.asarray(bv, np.float32)) + np.asarray(
        bo, np.float32
    )
    for b in range(B):
        acc = results[b * ng]["yT"].astype(np.float32)
        for g in range(1, ng):
            acc = acc + results[b * ng + g]["yT"].astype(np.float32)
        y[b] = acc.T + extra[None, :]
    return y


def kernel(x, wq, bq, wk, bk, wv, bv, wo, bo, mask, _trace=False):
    from concourse.bass_utils import run_bass_kernel_spmd

    causal = bool(np.asarray(mask).item())
    nc = get_nc(causal)
    in_maps = make_in_maps(x, wq, bq, wk, bk, wv, bv, wo, bo)
    res = run_bass_kernel_spmd(nc, in_maps, list(range(NCORES)), trace=_trace)
    y = combine_outputs(res.results, wo, bv, bo)
    if _trace:
        return y, res
    return y


# revision 4
# speedup vs baseline: 1.0107x; 1.0068x over previous
"""Trainium2 Bass kernel for 12-head causal MHA (B=2, S=2048, D=768), bf16 compute.

Sharding: 8 cores = (batch b in {0,1}) x (head-group hg in {0..3}, 3 heads each).

Per-core structure (per 512-column sequence block j):
  - v projection (x-stationary, natural [keys, vdim] layout, N=192)
  - q/k projections, w-stationary, in 4 groups (q01/k01 at 128 rows, q2/k2
    at 64 rows so each head's scores operands share a partition base);
    k bias is mathematically irrelevant under softmax and skipped, q bias
    applied via per-partition tensor_scalar on the PSUM->SBUF copy
  - scores [keys, q] per 128-key tile; full tiles exp'd in [128,1024] pairs,
    diagonal tiles exp'd at exact causal width then masked by a constant
    [128,512] 0/1 triangle (same relative pattern for every diagonal tile)
  - attn@V transposed: out[q, 65] = et^T @ v65 (65 = 64 vdims + ones col for
    the softmax denominator) -- free size 65 instead of 512 halves PE cost
  - normalize per-partition (query) via reciprocal of col 64 + broadcast mul
  - DMA-transpose (XBAR) ot [q, hd] -> otT [hd, q] SBUF->SBUF, heads 0,1
    packed on partitions 0..127 so the output projection contracts 192 dims
    in 2 K-groups
  - y^T partial = wo^T @ otT accumulated over 2 K-groups, copied to bf16 on
    DVE (GPSIMD cannot read PSUM on real HW), DMA'd out

Emission is globally software-pipelined with virtual PE/Act clocks: score+
exp units "feed" the Activation engine while attn@V chains, projections of
neighboring blocks, and the previous block's output projection drain as PE
filler whenever Act has backlog. The last block processes head 2 per
query-subtile so its transposes and output projection pipeline with the
remaining chains.

Host sums the 4 head-group partials per batch (fp32), transposes, and adds
bo + wo @ bv (bv is folded out of the device kernel).
"""

import math
from contextlib import ExitStack

import numpy as np

import concourse.bacc as bacc
import concourse.bass as bass
import concourse.mybir as mybir
import concourse.tile as tile

FP32 = mybir.dt.float32
BF16 = mybir.dt.bfloat16

B = 2
S = 2048
D = 768
NH = 12
DK = 64
NCORES = 8
HG = 3  # heads per core
HD = HG * DK  # 192
KC = D // 128  # 6 contraction chunks
SB = 512  # sequence block
NJ = S // SB  # 4
NT = S // 128  # 16 key tiles
SCALE = 1.0 / math.sqrt(DK)
EXP = mybir.ActivationFunctionType.Exp


def build_nc(causal: bool):
    nc = bacc.Bacc(trn_type="TRN2", target_bir_lowering=False, debug=False)

    x6_d = nc.declare_dram_parameter("x6", [128, KC, S], BF16, isOutput=False)
    wqk_d = nc.declare_dram_parameter("wqk", [128, KC, 3 * 128], BF16, isOutput=False)
    # wqk groups: g0 = wq heads 0,1 | g1 = wk heads 0,1 | g2 = [wq h2 | wk h2]
    wv6_d = nc.declare_dram_parameter("wv6", [128, KC, HD], BF16, isOutput=False)
    wo0_d = nc.declare_dram_parameter("wo0", [128, D], BF16, isOutput=False)
    wo1_d = nc.declare_dram_parameter("wo1", [64, D], BF16, isOutput=False)
    bqg_d = nc.declare_dram_parameter("bqg", [128, 2], FP32, isOutput=False)
    cm4_d = nc.declare_dram_parameter("cm4", [128, SB], BF16, isOutput=False)
    yT_d = nc.declare_dram_parameter("yT", [D, S], BF16, isOutput=True)

    with tile.TileContext(nc) as tc, ExitStack() as ctx:
        consts = ctx.enter_context(tc.tile_pool(name="consts", bufs=1))

        x6_sb = consts.tile([128, KC, S], BF16)
        wqk_sb = consts.tile([128, KC, 3 * 128], BF16)
        wv6_sb = consts.tile([128, KC, HD], BF16)
        wo0_sb = consts.tile([128, D], BF16)
        wo1_sb = consts.tile([64, D], BF16)
        bqg_sb = consts.tile([128, 2], FP32)
        cm4_sb = consts.tile([128, SB], BF16)
        qT01_sb = consts.tile([128, S], BF16)  # q heads 0,1
        kT01_sb = consts.tile([128, S], BF16)  # k heads 0,1
        qT2_sb = consts.tile([64, S], BF16)  # q head 2
        kT2_sb = consts.tile([64, S], BF16)  # k head 2
        v65_sb = consts.tile([128, NT, HG, 65], BF16)

        # ---- input DMAs: v weights (SWDGE path, parallel with HWDGE) + x
        # tile 0 first so compute starts early
        nc.gpsimd.dma_start(out=wv6_sb, in_=wv6_d.ap())
        nc.sync.dma_start(out=x6_sb[:, :, 0:128], in_=x6_d.ap()[:, :, 0:128])
        nc.sync.dma_start(out=x6_sb[:, :, 128:SB], in_=x6_d.ap()[:, :, 128:SB])
        nc.sync.dma_start(out=wqk_sb[:, :, 0:128], in_=wqk_d.ap()[:, :, 0:128])
        nc.sync.dma_start(out=wqk_sb[:, :, 128:256], in_=wqk_d.ap()[:, :, 128:256])
        nc.sync.dma_start(out=wqk_sb[:, :, 256:384], in_=wqk_d.ap()[:, :, 256:384])
        nc.scalar.dma_start(out=bqg_sb, in_=bqg_d.ap())
        if causal:
            nc.scalar.dma_start(out=cm4_sb, in_=cm4_d.ap())
        nc.sync.dma_start(
            out=x6_sb[:, :, SB : 2 * SB], in_=x6_d.ap()[:, :, SB : 2 * SB]
        )
        nc.scalar.dma_start(out=wo0_sb, in_=wo0_d.ap())
        nc.scalar.dma_start(out=wo1_sb, in_=wo1_d.ap())
        for j in range(2, NJ):
            eng = nc.sync if j < 3 else nc.scalar
            eng.dma_start(
                out=x6_sb[:, :, j * SB : (j + 1) * SB],
                in_=x6_d.ap()[:, :, j * SB : (j + 1) * SB],
            )

        # ones column for the softmax denominator
        nc.vector.memset(v65_sb[:, :, :, 64:65], 1.0)

        sp_pool = ctx.enter_context(tc.tile_pool(name="sp", bufs=2, space="PSUM"))
        pj_pool = ctx.enter_context(tc.tile_pool(name="pj", bufs=2, space="PSUM"))
        oq_pool = ctx.enter_context(tc.tile_pool(name="oq", bufs=2, space="PSUM"))
        et_pool = ctx.enter_context(tc.tile_pool(name="et", bufs=4))
        ot_pool = ctx.enter_context(tc.tile_pool(name="ot", bufs=2))
        rc_pool = ctx.enter_context(tc.tile_pool(name="rc", bufs=2))
        oT_pool = ctx.enter_context(tc.tile_pool(name="oT", bufs=2))
        yt_pool = ctx.enter_context(tc.tile_pool(name="yt", bufs=3))

        def q_ap(h, j):  # [64, SB] moving operand for scores
            src, base = (
                (qT01_sb, 0) if h == 0 else (qT01_sb, 64) if h == 1 else (qT2_sb, 0)
            )
            return src[base : base + 64, j * SB : (j + 1) * SB]

        def k_ap(h, t):  # [64, 128] stationary operand for scores
            src, base = (
                (kT01_sb, 0) if h == 0 else (kT01_sb, 64) if h == 1 else (kT2_sb, 0)
            )
            return src[base : base + 64, t * 128 : (t + 1) * 128]

        def projv_units(j):
            # v projection: x-stationary, per key tile, N=192
            for st in range(4 * j, 4 * (j + 1)):
                def unit(st=st):
                    vp = pj_pool.tile([128, SB], FP32, name="pj")
                    for c in range(KC):
                        nc.tensor.matmul(
                            vp[:, 0:HD],
                            lhsT=x6_sb[:, c, st * 128 : (st + 1) * 128],
                            rhs=wv6_sb[:, c, :],
                            start=(c == 0),
                            stop=(c == KC - 1),
                        )
                    nc.vector.tensor_copy(
                        v65_sb[:, st, :, 0:64],
                        vp[:, 0:HD].rearrange("p (h d) -> p h d", h=HG),
                    )
                yield unit

        def projqk_units(j, part=None):
            # q/k projections, w-stationary: two 128-row groups (q01, k01)
            # and two 64-row groups (q2, k2) so scores operands share a
            # partition base per head. part "a" = heads 0,1; "b" = head 2.
            jsp = slice(j * SB, (j + 1) * SB)
            groups = (
                (0, 128, 0, qT01_sb, bqg_sb[:, 0:1]),
                (1, 128, 0, kT01_sb, None),
                (2, 64, 0, qT2_sb, bqg_sb[0:64, 1:2]),
                (2, 64, 64, kT2_sb, None),
            )
            if part == "a":
                groups = groups[0:2]
            elif part == "b":
                groups = groups[2:4]
            for g, m, w0, dst, bias in groups:
                def unit(g=g, m=m, w0=w0, dst=dst, bias=bias):
                    pp = pj_pool.tile([128, SB], FP32, name="pj")
                    for c in range(KC):
                        nc.tensor.matmul(
                            pp[0:m, :],
                            lhsT=wqk_sb[:, c, g * 128 + w0 : g * 128 + w0 + m],
                            rhs=x6_sb[:, c, jsp],
                            start=(c == 0),
                            stop=(c == KC - 1),
                        )
                    if bias is not None:
                        nc.vector.tensor_scalar_add(dst[:, jsp], pp[0:m, :], bias)
                    else:
                        nc.vector.tensor_copy(dst[:, jsp], pp[0:m, :])
                yield unit

        def scores_full_units(j, h, et_t):
            """Full (off-diagonal) score tiles of one head, exp'd in pairs."""
            nfull = 4 * j if causal else NT
            for t0 in range(0, nfull, 2):
                def full_pair(t0=t0):
                    spf = sp_pool.tile([128, 2 * SB], FP32, name="sp")
                    for u in range(2):
                        nc.tensor.matmul(
                            spf[:, u * SB : (u + 1) * SB],
                            lhsT=k_ap(h, t0 + u),
                            rhs=q_ap(h, j),
                            start=True,
                            stop=True,
                        )
                    nc.scalar.activation(
                        et_t[:, t0 : t0 + 2, :], spf, EXP, scale=SCALE
                    )
                yield full_pair

        def scores_diag_units(j, h, et_t):
            """Diagonal score tiles at exact causal width, then 0/1 mask."""
            for u in range(4):
                def diag(u=u):
                    t = 4 * j + u
                    off = 128 * u
                    n = SB - off
                    spd = sp_pool.tile([128, 2 * SB], FP32, name="sp")
                    nc.tensor.matmul(
                        spd[:, 0:n],
                        lhsT=k_ap(h, t),
                        rhs=q_ap(h, j)[:, off:SB],
                        start=True,
                        stop=True,
                    )
                    nc.scalar.activation(
                        et_t[:, t, off:SB], spd[:, 0:n], EXP, scale=SCALE
                    )
                    nc.vector.tensor_mul(
                        et_t[:, t, off:SB],
                        et_t[:, t, off:SB],
                        cm4_sb[:, 0:n],
                    )
                yield diag

        def attnv_units(j, h, et_t, rc_t, ot4):
            """attn@V chains + normalize for one head."""
            tend = 4 * (j + 1) if causal else NT
            oq_t = oq_pool.tile([128, 4, 128], FP32)
            for qs in range(4):
                def chain(qs=qs):
                    tq = (4 * j + qs + 1) if causal else tend
                    for t in range(tq):
                        nc.tensor.matmul(
                            oq_t[:, qs, 0:65],
                            lhsT=et_t[:, t, qs * 128 : (qs + 1) * 128],
                            rhs=v65_sb[:, t, h, :],
                            start=(t == 0),
                            stop=(t == tq - 1),
                        )
                yield chain

            def normalize():
                # normalize immediately so the oq buffer frees early:
                # ot4[q, qs, h, :] = oq[q, qs, 0:64] / oq[q, qs, 64]
                nc.vector.reciprocal(rc_t[:, h * 4 : (h + 1) * 4], oq_t[:, :, 64])
                nc.vector.tensor_mul(
                    ot4[:, :, h, :],
                    oq_t[:, :, 0:64],
                    rc_t[:, h * 4 : (h + 1) * 4].unsqueeze(-1).to_broadcast(
                        (128, 4, 64)
                    ),
                )
            yield normalize

        def yproj_units(j, oT01, oT2, copy_engines):
            jsp = slice(j * SB, (j + 1) * SB)
            # output projection: 2 contraction groups (128 + 64)
            for dt in range(KC):
                def unit(dt=dt):
                    yp = pj_pool.tile([128, SB], FP32, name="pj")
                    nc.tensor.matmul(
                        yp,
                        lhsT=wo0_sb[:, dt * 128 : (dt + 1) * 128],
                        rhs=oT01,
                        start=True,
                        stop=False,
                    )
                    nc.tensor.matmul(
                        yp,
                        lhsT=wo1_sb[:, dt * 128 : (dt + 1) * 128],
                        rhs=oT2[0:64, :],
                        start=False,
                        stop=True,
                    )
                    yt = yt_pool.tile([128, SB], BF16)
                    eng = copy_engines[dt % len(copy_engines)]
                    if eng is nc.scalar:
                        eng.copy(yt, yp)
                    else:
                        eng.tensor_copy(yt, yp)
                    nc.sync.dma_start(
                        out=yT_d.ap()[dt * 128 : (dt + 1) * 128, jsp], in_=yt
                    )
                yield unit

        def interleave(feeder, filler):
            """Emit feeder units (which keep Act busy) with filler PE units
            spread evenly between them; leftover fillers go at the end."""
            feeder = list(feeder)
            filler = list(filler)
            nf = len(feeder)
            emitted = 0
            for i, f in enumerate(feeder):
                f()
                want = (i + 1) * len(filler) // nf if nf else len(filler)
                while emitted < want:
                    filler[emitted]()
                    emitted += 1
            while emitted < len(filler):
                filler[emitted]()
                emitted += 1

        def transposes01(ot4):
            # XBAR transposes for heads 0,1 (packed on partitions 0..127)
            oT01 = oT_pool.tile([128, SB], BF16, name="oT01")
            for qs in range(4):
                nc.sync.dma_start_transpose(
                    out=oT01[:, qs * 128 : (qs + 1) * 128], in_=ot4[:, qs, 0:2, :]
                )
            return oT01

        def transposes2(ot4):
            # XBAR transpose for head 2 (+pad rows, never consumed)
            oT2 = oT_pool.tile([128, SB], BF16, name="oT2")
            for qs in range(4):
                nc.scalar.dma_start_transpose(
                    out=oT2[:, qs * 128 : (qs + 1) * 128], in_=ot4[:, qs, 2:4, :]
                )
            return oT2

        def run(units):
            for u in units:
                u()

        # Software-pipelined global schedule driven by virtual PE/Act
        # clocks: score+exp units are "feeders" (they load both engines),
        # everything else is PE-only "filler". A feeder is emitted when the
        # Act backlog is small (sp pool depth limits PE run-ahead anyway);
        # fillers drain while Act chews. Fillers carry across phases.
        PE_CYC = 0.4167

        def fp_cost(_):  # full pair: 2 scores + [128,1024] exp
            return 2 * SB * PE_CYC, 1024 * 0.833 + 185

        def dg_cost(u):  # diag tile u: score + exp + mask
            n = SB - 128 * u
            return n * PE_CYC, n * 0.833 + 185

        class tposes:
            oT01 = None
            oT2 = None

        if causal:
            clocks = {"pe": 0.0, "act": 0.0}
            fillers = []

            def emit_feeder(u, pe, act):
                u()
                clocks["pe"] += pe
                clocks["act"] = max(clocks["act"], clocks["pe"]) + act

            def emit_filler():
                pe, u = fillers.pop(0)
                u()
                clocks["pe"] += pe

            def phase(feeders):
                for u, pe, act in feeders:
                    # drain fillers while Act has >1.4us of backlog
                    while fillers and clocks["act"] - clocks["pe"] > 1400:
                        emit_filler()
                    emit_feeder(u, pe, act)

            def add_fillers(units, pe_each):
                fillers.extend((pe_each, u) for u in units)

            pv0 = list(projv_units(0))
            run(pv0)
            for u in projqk_units(0, "a"):
                u()
                clocks["pe"] += 1280
            ets = {}

            def et(j, h):
                if (j, h) not in ets:
                    ets[(j, h)] = et_pool.tile([128, NT, SB], BF16, name="et")
                return ets[(j, h)]

            def feed_full(j, h):
                return [(u, *fp_cost(0)) for u in scores_full_units(j, h, et(j, h))]

            def feed_diag(j, h):
                return [
                    (u, *dg_cost(i))
                    for i, u in enumerate(scores_diag_units(j, h, et(j, h)))
                ]

            pqkb_done = {}

            def mark(j):
                def m():
                    pqkb_done[j] = True
                return m

            for j in range(NJ):
                rc_t = rc_pool.tile([128, HG * 4], FP32)
                ot4 = ot_pool.tile([128, 4, 4, 64], BF16)
                nc.vector.memset(ot4[:, :, 3, :], 0.0)
                if j == 0:
                    add_fillers(projqk_units(0, "b"), 1280)
                    add_fillers([mark(0)], 0)
                    phase(feed_full(0, 0) + feed_diag(0, 0))
                else:
                    phase(feed_diag(j, 0))
                av0 = list(attnv_units(j, 0, ets.pop((j, 0)), rc_t, ot4))
                add_fillers(av0, 27 * (4 * j + 3))
                phase(feed_full(j, 1) + feed_diag(j, 1))
                av1 = list(attnv_units(j, 1, ets.pop((j, 1)), rc_t, ot4))
                add_fillers(av1, 27 * (4 * j + 3))
                add_fillers(
                    [lambda ot4=ot4: setattr(tposes, "oT01", transposes01(ot4))], 0
                )
                # head-2 q/k of this block must be in SBUF before its scores
                while fillers and not pqkb_done.get(j, False):
                    emit_filler()
                phase(feed_full(j, 2) + feed_diag(j, 2))
                if j + 1 < NJ:
                    av2 = list(attnv_units(j, 2, ets.pop((j, 2)), rc_t, ot4))
                    # q01/k01 of the next block precede its scores; Act still
                    # has the h2-scores backlog to chew while PE projects
                    while fillers:
                        emit_filler()
                    for u in projqk_units(j + 1, "a"):
                        u()
                        clocks["pe"] += 1280
                    add_fillers(av2, 27 * (4 * j + 3))
                    add_fillers(
                        [lambda ot4=ot4: setattr(tposes, "oT2", transposes2(ot4))], 0
                    )
                    phase(feed_full(j + 1, 0))
                    add_fillers(projqk_units(j + 1, "b"), 1280)
                    add_fillers([mark(j + 1)], 0)
                    add_fillers(projv_units(j + 1), 480)
                    while fillers and tposes.oT2 is None:
                        emit_filler()
                    oT01, oT2 = tposes.oT01, tposes.oT2
                    tposes.oT01 = tposes.oT2 = None
                    add_fillers(
                        yproj_units(j, oT01, oT2, (nc.vector, nc.vector, nc.vector)),
                        426,
                    )
                else:
                    # ---- last block: process head 2 per query-subtile so
                    # the XBAR transposes and the output projection pipeline
                    # with the remaining attn@V chains
                    et_t = ets.pop((j, 2))
                    oq_t = oq_pool.tile([128, 4, 128], FP32)
                    oT2 = oT_pool.tile([128, SB], BF16, name="oT2")
                    for qs in range(4):
                        tq = 4 * j + qs + 1
                        for t in range(tq):
                            nc.tensor.matmul(
                                oq_t[:, qs, 0:65],
                                lhsT=et_t[:, t, qs * 128 : (qs + 1) * 128],
                                rhs=v65_sb[:, t, 2, :],
                                start=(t == 0),
                                stop=(t == tq - 1),
                            )
                        nc.vector.reciprocal(
                            rc_t[:, 8 + qs : 9 + qs], oq_t[:, qs, 64:65]
                        )
                        nc.vector.tensor_mul(
                            ot4[:, qs, 2, :],
                            oq_t[:, qs, 0:64],
                            rc_t[:, 8 + qs : 9 + qs].to_broadcast((128, 64)),
                        )
                        eng = nc.scalar if qs % 2 else nc.sync
                        eng.dma_start_transpose(
                            out=oT2[:, qs * 128 : (qs + 1) * 128],
                            in_=ot4[:, qs, 2:4, :],
                        )
                        if qs == 1:
                            while fillers:
                                emit_filler()
                    oT01 = tposes.oT01
                    tposes.oT01 = None
                    run(yproj_units(j, oT01, oT2, (nc.vector, nc.scalar, nc.vector)))
            while fillers:
                emit_filler()
        else:
            run(projqk_units(0))
            for j in range(NJ):
                run(projv_units(j))
                if j + 1 < NJ:
                    run(projqk_units(j + 1))
            yprev = []
            for j in range(NJ):
                rc_t = rc_pool.tile([128, HG * 4], FP32)
                ot4 = ot_pool.tile([128, 4, 4, 64], BF16)
                nc.vector.memset(ot4[:, :, 3, :], 0.0)
                ets = [
                    et_pool.tile([128, NT, SB], BF16, name="et") for _ in range(HG)
                ]
                interleave(scores_full_units(j, 0, ets[0]), yprev)
                interleave(
                    scores_full_units(j, 1, ets[1]),
                    attnv_units(j, 0, ets[0], rc_t, ot4),
                )
                interleave(
                    scores_full_units(j, 2, ets[2]),
                    attnv_units(j, 1, ets[1], rc_t, ot4),
                )
                run(attnv_units(j, 2, ets[2], rc_t, ot4))
                oT01 = transposes01(ot4)
                oT2 = transposes2(ot4)
                engines = (nc.vector, nc.vector, nc.vector)
                yprev = list(yproj_units(j, oT01, oT2, engines))
                for u in yprev:
                    u()
                yprev = []

    nc.finalize()
    return nc


_NC_CACHE: dict[bool, object] = {}


def get_nc(causal: bool):
    if causal not in _NC_CACHE:
        _NC_CACHE[causal] = build_nc(causal)
    return _NC_CACHE[causal]


def _bf16(a):
    import ml_dtypes

    return np.asarray(a, np.float32).astype(ml_dtypes.bfloat16)


def _chunked(wT):
    """[768, N] -> [128, 6, N] with chunk c = rows 128c..128c+127."""
    n = wT.shape[1]
    return np.ascontiguousarray(wT.reshape(KC, 128, n).transpose(1, 0, 2))


def _make_cm4():
    # relative triangle mask: cm4[p, c] = 1.0 iff c >= p (every diagonal
    # tile uses the width-n prefix of this pattern)
    p = np.arange(128)[:, None]
    c = np.arange(SB)[None, :]
    return (c >= p).astype(np.float32)


def make_in_maps(x, wq, bq, wk, bk, wv, bv, wo, bo):
    f32 = np.float32
    x = np.asarray(x, f32)
    cm4 = _bf16(_make_cm4())
    in_maps = []
    for core in range(NCORES):
        b, hg = divmod(core, NH // HG)
        hs = slice(hg * HD, (hg + 1) * HD)
        wqT = np.asarray(wq, f32)[hs, :].T  # [768, 192]
        wkT = np.asarray(wk, f32)[hs, :].T
        wqkT = np.concatenate(
            [wqT[:, 0:128], wkT[:, 0:128], wqT[:, 128:192], wkT[:, 128:192]], axis=1
        )
        bqc = np.asarray(bq, f32)[hs]
        bqg = np.zeros((128, 2), f32)
        bqg[:, 0] = bqc[0:128]
        bqg[0:64, 1] = bqc[128:192]
        woT = np.asarray(wo, f32)[:, hs].T  # [192, 768]
        in_maps.append(
            {
                "x6": _bf16(_chunked(np.ascontiguousarray(x[b].T))),
                "wqk": _bf16(_chunked(wqkT)),
                "wv6": _bf16(_chunked(np.asarray(wv, f32)[hs, :].T)),
                "wo0": _bf16(woT[0:128, :]),
                "wo1": _bf16(woT[128:192, :]),
                "bqg": bqg,
                "cm4": cm4,
            }
        )
    return in_maps


def combine_outputs(results, wo, bv, bo):
    y = np.empty((B, S, D), np.float32)
    ng = NH // HG
    extra = (np.asarray(wo, np.float32) @ np.asarray(bv, np.float32)) + np.asarray(
        bo, np.float32
    )
    for b in range(B):
        acc = results[b * ng]["yT"].astype(np.float32)
        for g in range(1, ng):
            acc = acc + results[b * ng + g]["yT"].astype(np.float32)
        y[b] = acc.T + extra[None, :]
    return y


def kernel(x, wq, bq, wk, bk, wv, bv, wo, bo, mask, _trace=False):
    from concourse.bass_utils import run_bass_kernel_spmd

    causal = bool(np.asarray(mask).item())
    nc = get_nc(causal)
    in_maps = make_in_maps(x, wq, bq, wk, bk, wv, bv, wo, bo)
    res = run_bass_kernel_spmd(nc, in_maps, list(range(NCORES)), trace=_trace)
    y = combine_outputs(res.results, wo, bv, bo)
    if _trace:
        return y, res
    return y


# revision 5
# speedup vs baseline: 1.0452x; 1.0341x over previous
"""Trainium2 Bass kernel for 12-head causal MHA (B=2, S=2048, D=768), bf16 compute.

Sharding: 8 cores = (batch b in {0,1}) x (head-group hg in {0..3}, 3 heads each).

Per-core structure (per 512-column sequence block j):
  - v projection (x-stationary, natural [keys, vdim] layout, N=192)
  - q/k projections, w-stationary, in 4 groups (q01/k01 at 128 rows, q2/k2
    at 64 rows so each head's scores operands share a partition base);
    k bias is mathematically irrelevant under softmax and skipped, q bias
    applied via per-partition tensor_scalar on the PSUM->SBUF copy
  - scores [keys, q] per 128-key tile; full tiles exp'd in [128,1024] pairs,
    diagonal tiles exp'd at exact causal width then masked by a constant
    [128,512] 0/1 triangle (same relative pattern for every diagonal tile)
  - attn@V transposed: out[q, 65] = et^T @ v65 (65 = 64 vdims + ones col for
    the softmax denominator) -- free size 65 instead of 512 halves PE cost
  - normalize per-partition (query) via reciprocal of col 64 + broadcast mul
  - DMA-transpose (XBAR) ot [q, hd] -> otT [hd, q] SBUF->SBUF, heads 0,1
    packed on partitions 0..127 so the output projection contracts 192 dims
    in 2 K-groups
  - y^T partial = wo^T @ otT accumulated over 2 K-groups, copied to bf16 on
    DVE (GPSIMD cannot read PSUM on real HW), DMA'd out

Emission is globally software-pipelined with virtual PE/Act clocks: score+
exp units "feed" the Activation engine while attn@V chains, projections of
neighboring blocks, and the previous block's output projection drain as PE
filler whenever Act has backlog. The last block processes head 2 per
query-subtile so its transposes and output projection pipeline with the
remaining chains.

Host sums the 4 head-group partials per batch (fp32), transposes, and adds
bo + wo @ bv (bv is folded out of the device kernel).
"""

import math
from contextlib import ExitStack

import numpy as np

import concourse.bacc as bacc
import concourse.bass as bass
import concourse.mybir as mybir
import concourse.tile as tile

FP32 = mybir.dt.float32
BF16 = mybir.dt.bfloat16

B = 2
S = 2048
D = 768
NH = 12
DK = 64
NCORES = 8
HG = 3  # heads per core
HD = HG * DK  # 192
KC = D // 128  # 6 contraction chunks
SB = 512  # sequence block
NJ = S // SB  # 4
NT = S // 128  # 16 key tiles
SCALE = 1.0 / math.sqrt(DK)
EXP = mybir.ActivationFunctionType.Exp


def build_nc(causal: bool):
    nc = bacc.Bacc(trn_type="TRN2", target_bir_lowering=False, debug=False)

    x6_d = nc.declare_dram_parameter("x6", [128, KC, S], BF16, isOutput=False)
    wqk_d = nc.declare_dram_parameter("wqk", [128, KC, 3 * 128], BF16, isOutput=False)
    # wqk groups: g0 = wq heads 0,1 | g1 = wk heads 0,1 | g2 = [wq h2 | wk h2]
    wv6_d = nc.declare_dram_parameter("wv6", [128, KC, HD], BF16, isOutput=False)
    wo0_d = nc.declare_dram_parameter("wo0", [128, D], BF16, isOutput=False)
    wo1_d = nc.declare_dram_parameter("wo1", [64, D], BF16, isOutput=False)
    bqg_d = nc.declare_dram_parameter("bqg", [128, 2], FP32, isOutput=False)
    cm4_d = nc.declare_dram_parameter("cm4", [128, SB], BF16, isOutput=False)
    yT_d = nc.declare_dram_parameter("yT", [D, S], BF16, isOutput=True)

    with tile.TileContext(nc) as tc, ExitStack() as ctx:
        consts = ctx.enter_context(tc.tile_pool(name="consts", bufs=1))

        x6_sb = consts.tile([128, KC, S], BF16)
        wqk_sb = consts.tile([128, KC, 3 * 128], BF16)
        wv6_sb = consts.tile([128, KC, HD], BF16)
        wo0_sb = consts.tile([128, D], BF16)
        wo1_sb = consts.tile([64, D], BF16)
        bqg_sb = consts.tile([128, 2], FP32)
        cm4_sb = consts.tile([128, SB], BF16)
        qT01_sb = consts.tile([128, S], BF16)  # q heads 0,1
        kT01_sb = consts.tile([128, S], BF16)  # k heads 0,1
        qT2_sb = consts.tile([64, S], BF16)  # q head 2
        kT2_sb = consts.tile([64, S], BF16)  # k head 2
        v65_sb = consts.tile([128, NT, HG, 65], BF16)

        # ---- input DMAs: v weights (SWDGE path, parallel with HWDGE) + x
        # tile 0 first so compute starts early
        nc.gpsimd.dma_start(out=wv6_sb, in_=wv6_d.ap())
        nc.sync.dma_start(out=x6_sb[:, :, 0:128], in_=x6_d.ap()[:, :, 0:128])
        nc.sync.dma_start(out=x6_sb[:, :, 128:SB], in_=x6_d.ap()[:, :, 128:SB])
        nc.sync.dma_start(out=wqk_sb[:, :, 0:128], in_=wqk_d.ap()[:, :, 0:128])
        nc.sync.dma_start(out=wqk_sb[:, :, 128:256], in_=wqk_d.ap()[:, :, 128:256])
        nc.sync.dma_start(out=wqk_sb[:, :, 256:384], in_=wqk_d.ap()[:, :, 256:384])
        nc.scalar.dma_start(out=bqg_sb, in_=bqg_d.ap())
        if causal:
            nc.scalar.dma_start(out=cm4_sb, in_=cm4_d.ap())
        nc.sync.dma_start(
            out=x6_sb[:, :, SB : 2 * SB], in_=x6_d.ap()[:, :, SB : 2 * SB]
        )
        nc.scalar.dma_start(out=wo0_sb, in_=wo0_d.ap())
        nc.scalar.dma_start(out=wo1_sb, in_=wo1_d.ap())
        for j in range(2, NJ):
            eng = nc.sync if j < 3 else nc.scalar
            eng.dma_start(
                out=x6_sb[:, :, j * SB : (j + 1) * SB],
                in_=x6_d.ap()[:, :, j * SB : (j + 1) * SB],
            )

        # ones column for the softmax denominator
        nc.vector.memset(v65_sb[:, :, :, 64:65], 1.0)

        sp_pool = ctx.enter_context(tc.tile_pool(name="sp", bufs=2, space="PSUM"))
        pj_pool = ctx.enter_context(tc.tile_pool(name="pj", bufs=2, space="PSUM"))
        oq_pool = ctx.enter_context(tc.tile_pool(name="oq", bufs=2, space="PSUM"))
        et_pool = ctx.enter_context(tc.tile_pool(name="et", bufs=5))
        ot_pool = ctx.enter_context(tc.tile_pool(name="ot", bufs=3))
        rc_pool = ctx.enter_context(tc.tile_pool(name="rc", bufs=2))
        oT_pool = ctx.enter_context(tc.tile_pool(name="oT", bufs=3))
        yt_pool = ctx.enter_context(tc.tile_pool(name="yt", bufs=6))

        def q_ap(h, j):  # [64, SB] moving operand for scores
            src, base = (
                (qT01_sb, 0) if h == 0 else (qT01_sb, 64) if h == 1 else (qT2_sb, 0)
            )
            return src[base : base + 64, j * SB : (j + 1) * SB]

        def k_ap(h, t):  # [64, 128] stationary operand for scores
            src, base = (
                (kT01_sb, 0) if h == 0 else (kT01_sb, 64) if h == 1 else (kT2_sb, 0)
            )
            return src[base : base + 64, t * 128 : (t + 1) * 128]

        def projv_units(j):
            # v projection: x-stationary, per key tile, N=192
            for st in range(4 * j, 4 * (j + 1)):
                def unit(st=st):
                    vp = pj_pool.tile([128, SB], FP32, name="pj")
                    for c in range(KC):
                        nc.tensor.matmul(
                            vp[:, 0:HD],
                            lhsT=x6_sb[:, c, st * 128 : (st + 1) * 128],
                            rhs=wv6_sb[:, c, :],
                            start=(c == 0),
                            stop=(c == KC - 1),
                        )
                    nc.vector.tensor_copy(
                        v65_sb[:, st, :, 0:64],
                        vp[:, 0:HD].rearrange("p (h d) -> p h d", h=HG),
                    )
                yield unit

        def projqk_units(j, part=None):
            # q/k projections, w-stationary: two 128-row groups (q01, k01)
            # and two 64-row groups (q2, k2) so scores operands share a
            # partition base per head. part "a" = heads 0,1; "b" = head 2.
            jsp = slice(j * SB, (j + 1) * SB)
            groups = (
                (0, 128, 0, qT01_sb, bqg_sb[:, 0:1]),
                (1, 128, 0, kT01_sb, None),
                (2, 64, 0, qT2_sb, bqg_sb[0:64, 1:2]),
                (2, 64, 64, kT2_sb, None),
            )
            if part == "a":
                groups = groups[0:2]
            elif part == "b":
                groups = groups[2:4]
            for g, m, w0, dst, bias in groups:
                def unit(g=g, m=m, w0=w0, dst=dst, bias=bias):
                    pp = pj_pool.tile([128, SB], FP32, name="pj")
                    for c in range(KC):
                        nc.tensor.matmul(
                            pp[0:m, :],
                            lhsT=wqk_sb[:, c, g * 128 + w0 : g * 128 + w0 + m],
                            rhs=x6_sb[:, c, jsp],
                            start=(c == 0),
                            stop=(c == KC - 1),
                        )
                    if bias is not None:
                        nc.vector.tensor_scalar_add(dst[:, jsp], pp[0:m, :], bias)
                    else:
                        nc.vector.tensor_copy(dst[:, jsp], pp[0:m, :])
                yield unit

        def scores_full_units(j, h, et_t):
            """Full (off-diagonal) score tiles of one head, exp'd in pairs."""
            nfull = 4 * j if causal else NT
            for t0 in range(0, nfull, 2):
                def full_pair(t0=t0):
                    spf = sp_pool.tile([128, 2 * SB], FP32, name="sp")
                    for u in range(2):
                        nc.tensor.matmul(
                            spf[:, u * SB : (u + 1) * SB],
                            lhsT=k_ap(h, t0 + u),
                            rhs=q_ap(h, j),
                            start=True,
                            stop=True,
                        )
                    nc.scalar.activation(
                        et_t[:, t0 : t0 + 2, :], spf, EXP, scale=SCALE
                    )
                yield full_pair

        def scores_diag_units(j, h, et_t):
            """Diagonal score tiles at exact causal width, then 0/1 mask."""
            for u in range(4):
                def diag(u=u):
                    t = 4 * j + u
                    off = 128 * u
                    n = SB - off
                    spd = sp_pool.tile([128, 2 * SB], FP32, name="sp")
                    nc.tensor.matmul(
                        spd[:, 0:n],
                        lhsT=k_ap(h, t),
                        rhs=q_ap(h, j)[:, off:SB],
                        start=True,
                        stop=True,
                    )
                    nc.scalar.activation(
                        et_t[:, t, off:SB], spd[:, 0:n], EXP, scale=SCALE
                    )
                    nc.vector.tensor_mul(
                        et_t[:, t, off:SB],
                        et_t[:, t, off:SB],
                        cm4_sb[:, 0:n],
                    )
                yield diag

        def attnv_units(j, h, et_t, rc_t, ot4):
            """attn@V chains + normalize for one head."""
            tend = 4 * (j + 1) if causal else NT
            oq_t = oq_pool.tile([128, 4, 128], FP32)
            for qs in range(4):
                def chain(qs=qs):
                    tq = (4 * j + qs + 1) if causal else tend
                    for t in range(tq):
                        nc.tensor.matmul(
                            oq_t[:, qs, 0:65],
                            lhsT=et_t[:, t, qs * 128 : (qs + 1) * 128],
                            rhs=v65_sb[:, t, h, :],
                            start=(t == 0),
                            stop=(t == tq - 1),
                        )
                yield chain

            def normalize():
                # normalize immediately so the oq buffer frees early:
                # ot4[q, qs, h, :] = oq[q, qs, 0:64] / oq[q, qs, 64]
                nc.vector.reciprocal(rc_t[:, h * 4 : (h + 1) * 4], oq_t[:, :, 64])
                nc.vector.tensor_mul(
                    ot4[:, :, h, :],
                    oq_t[:, :, 0:64],
                    rc_t[:, h * 4 : (h + 1) * 4].unsqueeze(-1).to_broadcast(
                        (128, 4, 64)
                    ),
                )
            yield normalize

        def yproj_units(j, oT01, oT2, copy_engines):
            jsp = slice(j * SB, (j + 1) * SB)
            # output projection: 2 contraction groups (128 + 64)
            for dt in range(KC):
                def unit(dt=dt):
                    yp = pj_pool.tile([128, SB], FP32, name="pj")
                    nc.tensor.matmul(
                        yp,
                        lhsT=wo0_sb[:, dt * 128 : (dt + 1) * 128],
                        rhs=oT01,
                        start=True,
                        stop=False,
                    )
                    nc.tensor.matmul(
                        yp,
                        lhsT=wo1_sb[:, dt * 128 : (dt + 1) * 128],
                        rhs=oT2[0:64, :],
                        start=False,
                        stop=True,
                    )
                    yt = yt_pool.tile([128, SB], BF16)
                    eng = copy_engines[dt % len(copy_engines)]
                    if eng is nc.scalar:
                        eng.copy(yt, yp)
                    else:
                        eng.tensor_copy(yt, yp)
                    nc.sync.dma_start(
                        out=yT_d.ap()[dt * 128 : (dt + 1) * 128, jsp], in_=yt
                    )
                yield unit

        def interleave(feeder, filler):
            """Emit feeder units (which keep Act busy) with filler PE units
            spread evenly between them; leftover fillers go at the end."""
            feeder = list(feeder)
            filler = list(filler)
            nf = len(feeder)
            emitted = 0
            for i, f in enumerate(feeder):
                f()
                want = (i + 1) * len(filler) // nf if nf else len(filler)
                while emitted < want:
                    filler[emitted]()
                    emitted += 1
            while emitted < len(filler):
                filler[emitted]()
                emitted += 1

        def transposes01(ot4):
            # XBAR transposes for heads 0,1 (packed on partitions 0..127)
            oT01 = oT_pool.tile([128, SB], BF16, name="oT01")
            for qs in range(4):
                nc.sync.dma_start_transpose(
                    out=oT01[:, qs * 128 : (qs + 1) * 128], in_=ot4[:, qs, 0:2, :]
                )
            return oT01

        def transposes2(ot4):
            # XBAR transpose for head 2 (+pad rows, never consumed)
            oT2 = oT_pool.tile([128, SB], BF16, name="oT2")
            for qs in range(4):
                nc.scalar.dma_start_transpose(
                    out=oT2[:, qs * 128 : (qs + 1) * 128], in_=ot4[:, qs, 2:4, :]
                )
            return oT2

        def run(units):
            for u in units:
                u()

        # Software-pipelined global schedule driven by virtual PE/Act
        # clocks: score+exp units are "feeders" (they load both engines),
        # everything else is PE-only "filler". A feeder is emitted when the
        # Act backlog is small (sp pool depth limits PE run-ahead anyway);
        # fillers drain while Act chews. Fillers carry across phases.
        PE_CYC = 0.4167

        def fp_cost(_):  # full pair: 2 scores + [128,1024] exp
            return 2 * SB * PE_CYC, 1024 * 0.833 + 185

        def dg_cost(u):  # diag tile u: score + exp + mask
            n = SB - 128 * u
            return n * PE_CYC, n * 0.833 + 185

        class tposes:
            oT01 = None
            oT2 = None

        if causal:
            clocks = {"pe": 0.0, "act": 0.0}
            fillers = []

            def emit_feeder(u, pe, act):
                u()
                clocks["pe"] += pe
                clocks["act"] = max(clocks["act"], clocks["pe"]) + act

            def emit_filler():
                pe, u = fillers.pop(0)
                u()
                clocks["pe"] += pe

            def phase(feeders):
                for u, pe, act in feeders:
                    # drain fillers while Act has >1.4us of backlog
                    while fillers and clocks["act"] - clocks["pe"] > 1400:
                        emit_filler()
                    emit_feeder(u, pe, act)

            def add_fillers(units, pe_each):
                fillers.extend((pe_each, u) for u in units)

            pv0 = list(projv_units(0))
            run(pv0)
            for u in projqk_units(0, "a"):
                u()
                clocks["pe"] += 1280
            ets = {}

            def et(j, h):
                if (j, h) not in ets:
                    ets[(j, h)] = et_pool.tile([128, NT, SB], BF16, name="et")
                return ets[(j, h)]

            def feed_full(j, h):
                return [(u, *fp_cost(0)) for u in scores_full_units(j, h, et(j, h))]

            def feed_diag(j, h):
                return [
                    (u, *dg_cost(i))
                    for i, u in enumerate(scores_diag_units(j, h, et(j, h)))
                ]

            pqkb_done = {}

            def mark(j):
                def m():
                    pqkb_done[j] = True
                return m

            for j in range(NJ):
                rc_t = rc_pool.tile([128, HG * 4], FP32)
                ot4 = ot_pool.tile([128, 4, 4, 64], BF16)
                nc.vector.memset(ot4[:, :, 3, :], 0.0)
                if j == 0:
                    add_fillers(projqk_units(0, "b"), 1280)
                    add_fillers([mark(0)], 0)
                    phase(feed_full(0, 0) + feed_diag(0, 0))
                else:
                    phase(feed_diag(j, 0))
                av0 = list(attnv_units(j, 0, ets.pop((j, 0)), rc_t, ot4))
                add_fillers(av0, 27 * (4 * j + 3))
                phase(feed_full(j, 1) + feed_diag(j, 1))
                av1 = list(attnv_units(j, 1, ets.pop((j, 1)), rc_t, ot4))
                add_fillers(av1, 27 * (4 * j + 3))
                add_fillers(
                    [lambda ot4=ot4: setattr(tposes, "oT01", transposes01(ot4))], 0
                )
                # head-2 q/k of this block must be in SBUF before its scores
                while fillers and not pqkb_done.get(j, False):
                    emit_filler()
                phase(feed_full(j, 2) + feed_diag(j, 2))
                if j + 1 < NJ:
                    av2 = list(attnv_units(j, 2, ets.pop((j, 2)), rc_t, ot4))
                    # q01/k01 of the next block precede its scores; Act still
                    # has the h2-scores backlog to chew while PE projects
                    while fillers:
                        emit_filler()
                    for u in projqk_units(j + 1, "a"):
                        u()
                        clocks["pe"] += 1280
                    add_fillers(av2, 27 * (4 * j + 3))
                    add_fillers(
                        [lambda ot4=ot4: setattr(tposes, "oT2", transposes2(ot4))], 0
                    )
                    phase(feed_full(j + 1, 0))
                    add_fillers(projqk_units(j + 1, "b"), 1280)
                    add_fillers([mark(j + 1)], 0)
                    add_fillers(projv_units(j + 1), 480)
                    while fillers and tposes.oT2 is None:
                        emit_filler()
                    oT01, oT2 = tposes.oT01, tposes.oT2
                    tposes.oT01 = tposes.oT2 = None
                    add_fillers(
                        yproj_units(j, oT01, oT2, (nc.vector, nc.vector, nc.vector)),
                        426,
                    )
                else:
                    # ---- last block: process head 2 per query-subtile so
                    # the XBAR transposes and the output projection pipeline
                    # with the remaining attn@V chains
                    et_t = ets.pop((j, 2))
                    oq_t = oq_pool.tile([128, 4, 128], FP32)
                    oT2 = oT_pool.tile([128, SB], BF16, name="oT2")
                    for qs in range(4):
                        tq = 4 * j + qs + 1
                        for t in range(tq):
                            nc.tensor.matmul(
                                oq_t[:, qs, 0:65],
                                lhsT=et_t[:, t, qs * 128 : (qs + 1) * 128],
                                rhs=v65_sb[:, t, 2, :],
                                start=(t == 0),
                                stop=(t == tq - 1),
                            )
                        nc.vector.reciprocal(
                            rc_t[:, 8 + qs : 9 + qs], oq_t[:, qs, 64:65]
                        )
                        nc.vector.tensor_mul(
                            ot4[:, qs, 2, :],
                            oq_t[:, qs, 0:64],
                            rc_t[:, 8 + qs : 9 + qs].to_broadcast((128, 64)),
                        )
                        eng = nc.scalar if qs % 2 else nc.sync
                        eng.dma_start_transpose(
                            out=oT2[:, qs * 128 : (qs + 1) * 128],
                            in_=ot4[:, qs, 2:4, :],
                        )
                        if qs == 1:
                            while fillers:
                                emit_filler()
                    oT01 = tposes.oT01
                    tposes.oT01 = None
                    run(yproj_units(j, oT01, oT2, (nc.vector, nc.scalar, nc.vector)))
            while fillers:
                emit_filler()
        else:
            run(projqk_units(0))
            for j in range(NJ):
                run(projv_units(j))
                if j + 1 < NJ:
                    run(projqk_units(j + 1))
            yprev = []
            for j in range(NJ):
                rc_t = rc_pool.tile([128, HG * 4], FP32)
                ot4 = ot_pool.tile([128, 4, 4, 64], BF16)
                nc.vector.memset(ot4[:, :, 3, :], 0.0)
                ets = [
                    et_pool.tile([128, NT, SB], BF16, name="et") for _ in range(HG)
                ]
                interleave(scores_full_units(j, 0, ets[0]), yprev)
                interleave(
                    scores_full_units(j, 1, ets[1]),
                    attnv_units(j, 0, ets[0], rc_t, ot4),
                )
                interleave(
                    scores_full_units(j, 2, ets[2]),
                    attnv_units(j, 1, ets[1], rc_t, ot4),
                )
                run(attnv_units(j, 2, ets[2], rc_t, ot4))
                oT01 = transposes01(ot4)
                oT2 = transposes2(ot4)
                engines = (nc.vector, nc.vector, nc.vector)
                yprev = list(yproj_units(j, oT01, oT2, engines))
                for u in yprev:
                    u()
                yprev = []

    nc.finalize()
    return nc


_NC_CACHE: dict[bool, object] = {}


def get_nc(causal: bool):
    if causal not in _NC_CACHE:
        _NC_CACHE[causal] = build_nc(causal)
    return _NC_CACHE[causal]


def _bf16(a):
    import ml_dtypes

    return np.asarray(a, np.float32).astype(ml_dtypes.bfloat16)


def _chunked(wT):
    """[768, N] -> [128, 6, N] with chunk c = rows 128c..128c+127."""
    n = wT.shape[1]
    return np.ascontiguousarray(wT.reshape(KC, 128, n).transpose(1, 0, 2))


def _make_cm4():
    # relative triangle mask: cm4[p, c] = 1.0 iff c >= p (every diagonal
    # tile uses the width-n prefix of this pattern)
    p = np.arange(128)[:, None]
    c = np.arange(SB)[None, :]
    return (c >= p).astype(np.float32)


def make_in_maps(x, wq, bq, wk, bk, wv, bv, wo, bo):
    f32 = np.float32
    x = np.asarray(x, f32)
    cm4 = _bf16(_make_cm4())
    in_maps = []
    for core in range(NCORES):
        b, hg = divmod(core, NH // HG)
        hs = slice(hg * HD, (hg + 1) * HD)
        wqT = np.asarray(wq, f32)[hs, :].T  # [768, 192]
        wkT = np.asarray(wk, f32)[hs, :].T
        wqkT = np.concatenate(
            [wqT[:, 0:128], wkT[:, 0:128], wqT[:, 128:192], wkT[:, 128:192]], axis=1
        )
        bqc = np.asarray(bq, f32)[hs]
        bqg = np.zeros((128, 2), f32)
        bqg[:, 0] = bqc[0:128]
        bqg[0:64, 1] = bqc[128:192]
        woT = np.asarray(wo, f32)[:, hs].T  # [192, 768]
        in_maps.append(
            {
                "x6": _bf16(_chunked(np.ascontiguousarray(x[b].T))),
                "wqk": _bf16(_chunked(wqkT)),
                "wv6": _bf16(_chunked(np.asarray(wv, f32)[hs, :].T)),
                "wo0": _bf16(woT[0:128, :]),
                "wo1": _bf16(woT[128:192, :]),
                "bqg": bqg,
                "cm4": cm4,
            }
        )
    return in_maps


def combine_outputs(results, wo, bv, bo):
    y = np.empty((B, S, D), np.float32)
    ng = NH // HG
    extra = (np.asarray(wo, np.float32) @ np.asarray(bv, np.float32)) + np.asarray(
        bo, np.float32
    )
    for b in range(B):
        acc = results[b * ng]["yT"].astype(np.float32)
        for g in range(1, ng):
            acc = acc + results[b * ng + g]["yT"].astype(np.float32)
        y[b] = acc.T + extra[None, :]
    return y


def kernel(x, wq, bq, wk, bk, wv, bv, wo, bo, mask, _trace=False):
    from concourse.bass_utils import run_bass_kernel_spmd

    causal = bool(np.asarray(mask).item())
    nc = get_nc(causal)
    in_maps = make_in_maps(x, wq, bq, wk, bk, wv, bv, wo, bo)
    res = run_bass_kernel_spmd(nc, in_maps, list(range(NCORES)), trace=_trace)
    y = combine_outputs(res.results, wo, bv, bo)
    if _trace:
        return y, res
    return y


# revision 6
# speedup vs baseline: 1.0461x; 1.0008x over previous
"""Trainium2 Bass kernel for 12-head causal MHA (B=2, S=2048, D=768), bf16 compute.

Sharding: 8 cores = (batch b in {0,1}) x (head-group hg in {0..3}, 3 heads each).

Per-core structure (per 512-column sequence block j):
  - v projection (x-stationary, natural [keys, vdim] layout, N=192)
  - q/k projections, w-stationary, in 4 groups (q01/k01 at 128 rows, q2/k2
    at 64 rows so each head's scores operands share a partition base);
    k bias is mathematically irrelevant under softmax and skipped, q bias
    applied via per-partition tensor_scalar on the PSUM->SBUF copy
  - scores [keys, q] per 128-key tile; full tiles exp'd in [128,1024] pairs,
    diagonal tiles exp'd at exact causal width then masked by a constant
    [128,512] 0/1 triangle (same relative pattern for every diagonal tile)
  - attn@V transposed: out[q, 65] = et^T @ v65 (65 = 64 vdims + ones col for
    the softmax denominator) -- free size 65 instead of 512 halves PE cost
  - normalize per-partition (query) via reciprocal of col 64 + broadcast mul
  - DMA-transpose (XBAR) ot [q, hd] -> otT [hd, q] SBUF->SBUF, heads 0,1
    packed on partitions 0..127 so the output projection contracts 192 dims
    in 2 K-groups
  - y^T partial = wo^T @ otT accumulated over 2 K-groups, copied to bf16 on
    DVE (GPSIMD cannot read PSUM on real HW), DMA'd out

Emission is globally software-pipelined with virtual PE/Act clocks: score+
exp units "feed" the Activation engine while attn@V chains, projections of
neighboring blocks, and the previous block's output projection drain as PE
filler whenever Act has backlog. The last block processes head 2 per
query-subtile so its transposes and output projection pipeline with the
remaining chains.

Host sums the 4 head-group partials per batch (fp32), transposes, and adds
bo + wo @ bv (bv is folded out of the device kernel).
"""

import math
from contextlib import ExitStack

import numpy as np

import concourse.bacc as bacc
import concourse.bass as bass
import concourse.mybir as mybir
import concourse.tile as tile

FP32 = mybir.dt.float32
BF16 = mybir.dt.bfloat16

B = 2
S = 2048
D = 768
NH = 12
DK = 64
NCORES = 8
HG = 3  # heads per core
HD = HG * DK  # 192
KC = D // 128  # 6 contraction chunks
SB = 512  # sequence block
NJ = S // SB  # 4
NT = S // 128  # 16 key tiles
SCALE = 1.0 / math.sqrt(DK)
EXP = mybir.ActivationFunctionType.Exp


def build_nc(causal: bool):
    nc = bacc.Bacc(trn_type="TRN2", target_bir_lowering=False, debug=False)

    x6_d = nc.declare_dram_parameter("x6", [128, KC, S], BF16, isOutput=False)
    wqk_d = nc.declare_dram_parameter("wqk", [128, KC, 3 * 128], BF16, isOutput=False)
    # wqk groups: g0 = wq heads 0,1 | g1 = wk heads 0,1 | g2 = [wq h2 | wk h2]
    wv6_d = nc.declare_dram_parameter("wv6", [128, KC, HD], BF16, isOutput=False)
    wo0_d = nc.declare_dram_parameter("wo0", [128, D], BF16, isOutput=False)
    wo1_d = nc.declare_dram_parameter("wo1", [64, D], BF16, isOutput=False)
    bqg_d = nc.declare_dram_parameter("bqg", [128, 2], FP32, isOutput=False)
    cm4_d = nc.declare_dram_parameter("cm4", [128, 1280], BF16, isOutput=False)
    yT_d = nc.declare_dram_parameter("yT", [D, S], BF16, isOutput=True)

    with tile.TileContext(nc) as tc, ExitStack() as ctx:
        consts = ctx.enter_context(tc.tile_pool(name="consts", bufs=1))

        x6_sb = consts.tile([128, KC, S], BF16)
        wqk_sb = consts.tile([128, KC, 3 * 128], BF16)
        wv6_sb = consts.tile([128, KC, HD], BF16)
        wo0_sb = consts.tile([128, D], BF16)
        wo1_sb = consts.tile([64, D], BF16)
        bqg_sb = consts.tile([128, 2], FP32)
        cm4_sb = consts.tile([128, 1280], BF16)
        qT01_sb = consts.tile([128, S], BF16)  # q heads 0,1
        kT01_sb = consts.tile([128, S], BF16)  # k heads 0,1
        qk2_sb = consts.tile([128, S], BF16)  # rows 0:64 q h2, 64:128 k h2
        kT2_sb = consts.tile([64, S], BF16)  # k head 2 (partition-hopped)
        v65_sb = consts.tile([128, NT, HG, 65], BF16)

        # ---- input DMAs: v weights (SWDGE path, parallel with HWDGE) + x
        # tile 0 first so compute starts early
        nc.gpsimd.dma_start(out=wv6_sb, in_=wv6_d.ap())
        nc.sync.dma_start(out=x6_sb[:, :, 0:128], in_=x6_d.ap()[:, :, 0:128])
        nc.sync.dma_start(out=x6_sb[:, :, 128:SB], in_=x6_d.ap()[:, :, 128:SB])
        nc.sync.dma_start(out=wqk_sb[:, :, 0:128], in_=wqk_d.ap()[:, :, 0:128])
        nc.sync.dma_start(out=wqk_sb[:, :, 128:256], in_=wqk_d.ap()[:, :, 128:256])
        nc.sync.dma_start(out=wqk_sb[:, :, 256:384], in_=wqk_d.ap()[:, :, 256:384])
        nc.scalar.dma_start(out=bqg_sb, in_=bqg_d.ap())
        if causal:
            nc.scalar.dma_start(out=cm4_sb, in_=cm4_d.ap())
        nc.sync.dma_start(
            out=x6_sb[:, :, SB : 2 * SB], in_=x6_d.ap()[:, :, SB : 2 * SB]
        )
        nc.scalar.dma_start(out=wo0_sb, in_=wo0_d.ap())
        nc.scalar.dma_start(out=wo1_sb, in_=wo1_d.ap())
        for j in range(2, NJ):
            eng = nc.sync if j < 3 else nc.scalar
            eng.dma_start(
                out=x6_sb[:, :, j * SB : (j + 1) * SB],
                in_=x6_d.ap()[:, :, j * SB : (j + 1) * SB],
            )

        # ones column for the softmax denominator
        nc.vector.memset(v65_sb[:, :, :, 64:65], 1.0)

        sp_pool = ctx.enter_context(tc.tile_pool(name="sp", bufs=2, space="PSUM"))
        pj_pool = ctx.enter_context(tc.tile_pool(name="pj", bufs=2, space="PSUM"))
        oq_pool = ctx.enter_context(tc.tile_pool(name="oq", bufs=2, space="PSUM"))
        et_pool = ctx.enter_context(tc.tile_pool(name="et", bufs=5))
        ot_pool = ctx.enter_context(tc.tile_pool(name="ot", bufs=3))
        rc_pool = ctx.enter_context(tc.tile_pool(name="rc", bufs=2))
        oT_pool = ctx.enter_context(tc.tile_pool(name="oT", bufs=3))
        yt_pool = ctx.enter_context(tc.tile_pool(name="yt", bufs=6))

        def q_ap(h, j):  # [64, SB] moving operand for scores
            src, base = (
                (qT01_sb, 0) if h == 0 else (qT01_sb, 64) if h == 1 else (qk2_sb, 0)
            )
            return src[base : base + 64, j * SB : (j + 1) * SB]

        def k_ap(h, t):  # [64, 128] stationary operand for scores
            src, base = (
                (kT01_sb, 0) if h == 0 else (kT01_sb, 64) if h == 1 else (kT2_sb, 0)
            )
            return src[base : base + 64, t * 128 : (t + 1) * 128]

        def projv_units(j):
            # v projection: x-stationary, per key tile, N=192
            for st in range(4 * j, 4 * (j + 1)):
                def unit(st=st):
                    vp = pj_pool.tile([128, SB], FP32, name="pj")
                    for c in range(KC):
                        nc.tensor.matmul(
                            vp[:, 0:HD],
                            lhsT=x6_sb[:, c, st * 128 : (st + 1) * 128],
                            rhs=wv6_sb[:, c, :],
                            start=(c == 0),
                            stop=(c == KC - 1),
                        )
                    nc.vector.tensor_copy(
                        v65_sb[:, st, :, 0:64],
                        vp[:, 0:HD].rearrange("p (h d) -> p h d", h=HG),
                    )
                yield unit

        def projqk_units(j, part=None):
            # q/k projections, w-stationary: two 128-row groups (q01, k01)
            # and two 64-row groups (q2, k2) so scores operands share a
            # partition base per head. part "a" = heads 0,1; "b" = head 2.
            jsp = slice(j * SB, (j + 1) * SB)
            groups = (
                (0, qT01_sb, bqg_sb[:, 0:1]),
                (1, kT01_sb, None),
                (2, qk2_sb, bqg_sb[:, 1:2]),
            )
            if part == "a":
                groups = groups[0:2]
            elif part == "b":
                groups = groups[2:3]
            for g, dst, bias in groups:
                def unit(g=g, dst=dst, bias=bias):
                    pp = pj_pool.tile([128, SB], FP32, name="pj")
                    for c in range(KC):
                        nc.tensor.matmul(
                            pp,
                            lhsT=wqk_sb[:, c, g * 128 : (g + 1) * 128],
                            rhs=x6_sb[:, c, jsp],
                            start=(c == 0),
                            stop=(c == KC - 1),
                        )
                    if bias is not None:
                        nc.vector.tensor_scalar_add(dst[:, jsp], pp, bias)
                    else:
                        nc.vector.tensor_copy(dst[:, jsp], pp)
                    if g == 2:
                        # k2 lives in rows 64:128 of qk2; hop to partitions
                        # 0:64 so scores h2 operands share a partition base
                        nc.gpsimd.dma_start(
                            out=kT2_sb[:, jsp], in_=dst[64:128, jsp]
                        )
                yield unit

        def scores_full_units(j, h, et_t):
            """Full (off-diagonal) score tiles of one head, exp'd in pairs."""
            nfull = 4 * j if causal else NT
            for t0 in range(0, nfull, 2):
                def full_pair(t0=t0):
                    spf = sp_pool.tile([128, 2 * SB], FP32, name="sp")
                    for u in range(2):
                        nc.tensor.matmul(
                            spf[:, u * SB : (u + 1) * SB],
                            lhsT=k_ap(h, t0 + u),
                            rhs=q_ap(h, j),
                            start=True,
                            stop=True,
                        )
                    nc.scalar.activation(
                        et_t[:, t0 : t0 + 2, :], spf, EXP, scale=SCALE
                    )
                yield full_pair

        # packed-diagonal layout: tile 4j+0 (full width) stays in its
        # normal et slot; tiles 4j+1..3 (widths 384/256/128) are packed
        # back-to-back into the 768 columns starting at et slot 4j+1, so
        # one exp + one mask-mul covers all three.
        DSEG = (0, 0, 384, 640)  # packed base offset of diag tile u (u>=1)

        def av_lhsT(et_t, j, t, qs):
            u = t - 4 * j
            if not causal or u < 1:
                return et_t[:, t, qs * 128 : (qs + 1) * 128]
            flat = et_t[:, :, :].rearrange("p a b -> p (a b)")
            base = (4 * j + 1) * SB + DSEG[u] + qs * 128 - 128 * u
            return flat[:, base : base + 128]

        def scores_diag_units(j, h, et_t):
            """Diagonal score tiles at exact causal width, then 0/1 mask."""
            def diag0():
                t = 4 * j
                spd = sp_pool.tile([128, 2 * SB], FP32, name="sp")
                nc.tensor.matmul(
                    spd[:, 0:SB],
                    lhsT=k_ap(h, t),
                    rhs=q_ap(h, j),
                    start=True,
                    stop=True,
                )
                nc.scalar.activation(et_t[:, t, :], spd[:, 0:SB], EXP, scale=SCALE)
                nc.vector.tensor_mul(
                    et_t[:, t, :], et_t[:, t, :], cm4_sb[:, 0:SB]
                )
            yield diag0

            def diag123():
                # PSUM segments offset so no matmul write crosses a 2KB
                # bank boundary: u1 at [128:512], u2 [512:768], u3 [768:896];
                # the exp reads the contiguous [128:896] span and writes the
                # gapless packed et region
                spd = sp_pool.tile([128, 2 * SB], FP32, name="sp")
                for u in range(1, 4):
                    t = 4 * j + u
                    off = 128 * u
                    n = SB - off
                    pseg = (0, 128, 512, 768)[u]
                    nc.tensor.matmul(
                        spd[:, pseg : pseg + n],
                        lhsT=k_ap(h, t),
                        rhs=q_ap(h, j)[:, off:SB],
                        start=True,
                        stop=True,
                    )
                flat = et_t[:, :, :].rearrange("p a b -> p (a b)")
                dst = flat[:, (4 * j + 1) * SB : (4 * j + 1) * SB + 768]
                nc.scalar.activation(dst, spd[:, 128:896], EXP, scale=SCALE)
                nc.vector.tensor_mul(dst, dst, cm4_sb[:, SB : SB + 768])
            yield diag123

        def attnv_units(j, h, et_t, rc_t, ot4):
            """attn@V chains + normalize for one head."""
            tend = 4 * (j + 1) if causal else NT
            oq_t = oq_pool.tile([128, 4, 128], FP32)
            for qs in range(4):
                def chain(qs=qs):
                    tq = (4 * j + qs + 1) if causal else tend
                    for t in range(tq):
                        nc.tensor.matmul(
                            oq_t[:, qs, 0:65],
                            lhsT=av_lhsT(et_t, j, t, qs),
                            rhs=v65_sb[:, t, h, :],
                            start=(t == 0),
                            stop=(t == tq - 1),
                        )
                yield chain

            def normalize():
                # normalize immediately so the oq buffer frees early:
                # ot4[q, qs, h, :] = oq[q, qs, 0:64] / oq[q, qs, 64]
                nc.vector.reciprocal(rc_t[:, h * 4 : (h + 1) * 4], oq_t[:, :, 64])
                nc.vector.tensor_mul(
                    ot4[:, :, h, :],
                    oq_t[:, :, 0:64],
                    rc_t[:, h * 4 : (h + 1) * 4].unsqueeze(-1).to_broadcast(
                        (128, 4, 64)
                    ),
                )
            yield normalize

        def yproj_units(j, oT01, oT2, copy_engines):
            jsp = slice(j * SB, (j + 1) * SB)
            # output projection: 2 contraction groups (128 + 64)
            for dt in range(KC):
                def unit(dt=dt):
                    yp = pj_pool.tile([128, SB], FP32, name="pj")
                    nc.tensor.matmul(
                        yp,
                        lhsT=wo0_sb[:, dt * 128 : (dt + 1) * 128],
                        rhs=oT01,
                        start=True,
                        stop=False,
                    )
                    nc.tensor.matmul(
                        yp,
                        lhsT=wo1_sb[:, dt * 128 : (dt + 1) * 128],
                        rhs=oT2[0:64, :],
                        start=False,
                        stop=True,
                    )
                    yt = yt_pool.tile([128, SB], BF16)
                    eng = copy_engines[dt % len(copy_engines)]
                    if eng is nc.scalar:
                        eng.copy(yt, yp)
                    else:
                        eng.tensor_copy(yt, yp)
                    nc.sync.dma_start(
                        out=yT_d.ap()[dt * 128 : (dt + 1) * 128, jsp], in_=yt
                    )
                yield unit

        def interleave(feeder, filler):
            """Emit feeder units (which keep Act busy) with filler PE units
            spread evenly between them; leftover fillers go at the end."""
            feeder = list(feeder)
            filler = list(filler)
            nf = len(feeder)
            emitted = 0
            for i, f in enumerate(feeder):
                f()
                want = (i + 1) * len(filler) // nf if nf else len(filler)
                while emitted < want:
                    filler[emitted]()
                    emitted += 1
            while emitted < len(filler):
                filler[emitted]()
                emitted += 1

        def transposes01(ot4):
            # XBAR transposes for heads 0,1 (packed on partitions 0..127)
            oT01 = oT_pool.tile([128, SB], BF16, name="oT01")
            for qs in range(4):
                nc.sync.dma_start_transpose(
                    out=oT01[:, qs * 128 : (qs + 1) * 128], in_=ot4[:, qs, 0:2, :]
                )
            return oT01

        def transposes2(ot4):
            # XBAR transpose for head 2 (+pad rows, never consumed)
            oT2 = oT_pool.tile([128, SB], BF16, name="oT2")
            for qs in range(4):
                nc.sync.dma_start_transpose(
                    out=oT2[:, qs * 128 : (qs + 1) * 128], in_=ot4[:, qs, 2:4, :]
                )
            return oT2

        def run(units):
            for u in units:
                u()

        # Software-pipelined global schedule driven by virtual PE/Act
        # clocks: score+exp units are "feeders" (they load both engines),
        # everything else is PE-only "filler". A feeder is emitted when the
        # Act backlog is small (sp pool depth limits PE run-ahead anyway);
        # fillers drain while Act chews. Fillers carry across phases.
        PE_CYC = 0.4167

        def fp_cost(_):  # full pair: 2 scores + [128,1024] exp
            return 2 * SB * PE_CYC, 1024 * 0.833 + 185

        def dg_cost(u):  # diag tile u: score + exp + mask
            n = SB - 128 * u
            return n * PE_CYC, n * 0.833 + 185

        class tposes:
            oT01 = None
            oT2 = None

        if causal:
            clocks = {"pe": 0.0, "act": 0.0}
            fillers = []

            def emit_feeder(u, pe, act):
                u()
                clocks["pe"] += pe
                clocks["act"] = max(clocks["act"], clocks["pe"]) + act

            def emit_filler():
                pe, u = fillers.pop(0)
                u()
                clocks["pe"] += pe

            def phase(feeders):
                for u, pe, act in feeders:
                    # drain fillers while Act has >1.4us of backlog
                    while fillers and clocks["act"] - clocks["pe"] > 1400:
                        emit_filler()
                    emit_feeder(u, pe, act)

            def add_fillers(units, pe_each):
                fillers.extend((pe_each, u) for u in units)

            pv0 = list(projv_units(0))
            run(pv0)
            for u in projqk_units(0, "a"):
                u()
                clocks["pe"] += 1280
            ets = {}

            def et(j, h):
                if (j, h) not in ets:
                    ets[(j, h)] = et_pool.tile([128, NT, SB], BF16, name="et")
                return ets[(j, h)]

            def feed_full(j, h):
                return [(u, *fp_cost(0)) for u in scores_full_units(j, h, et(j, h))]

            def feed_diag(j, h):
                costs = [(SB * 0.4167, 612.0), (768 * 0.4167, 825.0)]
                return [
                    (u, *costs[i])
                    for i, u in enumerate(scores_diag_units(j, h, et(j, h)))
                ]

            pqkb_done = {}

            def mark(j):
                def m():
                    pqkb_done[j] = True
                return m

            for j in range(NJ):
                rc_t = rc_pool.tile([128, HG * 4], FP32)
                ot4 = ot_pool.tile([128, 4, 4, 64], BF16)
                nc.vector.memset(ot4[:, :, 3, :], 0.0)
                if j == 0:
                    add_fillers(projqk_units(0, "b"), 1280)
                    add_fillers([mark(0)], 0)
                    phase(feed_full(0, 0) + feed_diag(0, 0))
                else:
                    phase(feed_diag(j, 0))
                av0 = list(attnv_units(j, 0, ets.pop((j, 0)), rc_t, ot4))
                add_fillers(av0, 27 * (4 * j + 3))
                phase(feed_full(j, 1) + feed_diag(j, 1))
                av1 = list(attnv_units(j, 1, ets.pop((j, 1)), rc_t, ot4))
                add_fillers(av1, 27 * (4 * j + 3))
                add_fillers(
                    [lambda ot4=ot4: setattr(tposes, "oT01", transposes01(ot4))], 0
                )
                # head-2 q/k of this block must be in SBUF before its scores
                while fillers and not pqkb_done.get(j, False):
                    emit_filler()
                phase(feed_full(j, 2) + feed_diag(j, 2))
                if j + 1 < NJ:
                    av2 = list(attnv_units(j, 2, ets.pop((j, 2)), rc_t, ot4))
                    # q01/k01 of the next block precede its scores; Act still
                    # has the h2-scores backlog to chew while PE projects
                    while fillers:
                        emit_filler()
                    for u in projqk_units(j + 1, "a"):
                        u()
                        clocks["pe"] += 1280
                    add_fillers(av2, 27 * (4 * j + 3))
                    add_fillers(
                        [lambda ot4=ot4: setattr(tposes, "oT2", transposes2(ot4))], 0
                    )
                    phase(feed_full(j + 1, 0))
                    add_fillers(projqk_units(j + 1, "b"), 1280)
                    add_fillers([mark(j + 1)], 0)
                    add_fillers(projv_units(j + 1), 480)
                    while fillers and tposes.oT2 is None:
                        emit_filler()
                    oT01, oT2 = tposes.oT01, tposes.oT2
                    tposes.oT01 = tposes.oT2 = None
                    add_fillers(
                        yproj_units(j, oT01, oT2, (nc.vector, nc.vector, nc.vector)),
                        426,
                    )
                else:
                    # ---- last block: process head 2 per query-subtile so
                    # the XBAR transposes and the output projection pipeline
                    # with the remaining attn@V chains
                    et_t = ets.pop((j, 2))
                    oq_t = oq_pool.tile([128, 4, 128], FP32)
                    oT2 = oT_pool.tile([128, SB], BF16, name="oT2")
                    for qs in range(4):
                        tq = 4 * j + qs + 1
                        for t in range(tq):
                            nc.tensor.matmul(
                                oq_t[:, qs, 0:65],
                                lhsT=av_lhsT(et_t, j, t, qs),
                                rhs=v65_sb[:, t, 2, :],
                                start=(t == 0),
                                stop=(t == tq - 1),
                            )
                        nc.vector.reciprocal(
                            rc_t[:, 8 + qs : 9 + qs], oq_t[:, qs, 64:65]
                        )
                        nc.vector.tensor_mul(
                            ot4[:, qs, 2, :],
                            oq_t[:, qs, 0:64],
                            rc_t[:, 8 + qs : 9 + qs].to_broadcast((128, 64)),
                        )
                        eng = nc.scalar if qs % 2 else nc.sync
                        eng.dma_start_transpose(
                            out=oT2[:, qs * 128 : (qs + 1) * 128],
                            in_=ot4[:, qs, 2:4, :],
                        )
                        if qs == 1:
                            while fillers:
                                emit_filler()
                    oT01 = tposes.oT01
                    tposes.oT01 = None
                    run(yproj_units(j, oT01, oT2, (nc.vector, nc.scalar, nc.vector)))
            while fillers:
                emit_filler()
        else:
            run(projqk_units(0))
            for j in range(NJ):
                run(projv_units(j))
                if j + 1 < NJ:
                    run(projqk_units(j + 1))
            yprev = []
            for j in range(NJ):
                rc_t = rc_pool.tile([128, HG * 4], FP32)
                ot4 = ot_pool.tile([128, 4, 4, 64], BF16)
                nc.vector.memset(ot4[:, :, 3, :], 0.0)
                ets = [
                    et_pool.tile([128, NT, SB], BF16, name="et") for _ in range(HG)
                ]
                interleave(scores_full_units(j, 0, ets[0]), yprev)
                interleave(
                    scores_full_units(j, 1, ets[1]),
                    attnv_units(j, 0, ets[0], rc_t, ot4),
                )
                interleave(
                    scores_full_units(j, 2, ets[2]),
                    attnv_units(j, 1, ets[1], rc_t, ot4),
                )
                run(attnv_units(j, 2, ets[2], rc_t, ot4))
                oT01 = transposes01(ot4)
                oT2 = transposes2(ot4)
                engines = (nc.vector, nc.vector, nc.vector)
                yprev = list(yproj_units(j, oT01, oT2, engines))
                for u in yprev:
                    u()
                yprev = []

    nc.finalize()
    return nc


_NC_CACHE: dict[bool, object] = {}


def get_nc(causal: bool):
    if causal not in _NC_CACHE:
        _NC_CACHE[causal] = build_nc(causal)
    return _NC_CACHE[causal]


def _bf16(a):
    import ml_dtypes

    return np.asarray(a, np.float32).astype(ml_dtypes.bfloat16)


def _chunked(wT):
    """[768, N] -> [128, 6, N] with chunk c = rows 128c..128c+127."""
    n = wT.shape[1]
    return np.ascontiguousarray(wT.reshape(KC, 128, n).transpose(1, 0, 2))


def _make_cm4():
    # packed relative triangle masks [tri512 | tri384 | tri256 | tri128]:
    # tri(n)[p, c] = 1.0 iff c >= p; diagonal tile u uses tri(512 - 128u)
    p = np.arange(128)[:, None]
    segs = []
    for n in (512, 384, 256, 128):
        c = np.arange(n)[None, :]
        segs.append((c >= p).astype(np.float32))
    return np.concatenate(segs, axis=1)


def make_in_maps(x, wq, bq, wk, bk, wv, bv, wo, bo):
    f32 = np.float32
    x = np.asarray(x, f32)
    cm4 = _bf16(_make_cm4())
    in_maps = []
    for core in range(NCORES):
        b, hg = divmod(core, NH // HG)
        hs = slice(hg * HD, (hg + 1) * HD)
        wqT = np.asarray(wq, f32)[hs, :].T  # [768, 192]
        wkT = np.asarray(wk, f32)[hs, :].T
        wqkT = np.concatenate(
            [wqT[:, 0:128], wkT[:, 0:128], wqT[:, 128:192], wkT[:, 128:192]], axis=1
        )
        bqc = np.asarray(bq, f32)[hs]
        bqg = np.zeros((128, 2), f32)
        bqg[:, 0] = bqc[0:128]
        bqg[0:64, 1] = bqc[128:192]
        woT = np.asarray(wo, f32)[:, hs].T  # [192, 768]
        in_maps.append(
            {
                "x6": _bf16(_chunked(np.ascontiguousarray(x[b].T))),
                "wqk": _bf16(_chunked(wqkT)),
                "wv6": _bf16(_chunked(np.asarray(wv, f32)[hs, :].T)),
                "wo0": _bf16(woT[0:128, :]),
                "wo1": _bf16(woT[128:192, :]),
                "bqg": bqg,
                "cm4": cm4,
            }
        )
    return in_maps


def combine_outputs(results, wo, bv, bo):
    y = np.empty((B, S, D), np.float32)
    ng = NH // HG
    extra = (np.asarray(wo, np.float32) @ np.asarray(bv, np.float32)) + np.asarray(
        bo, np.float32
    )
    for b in range(B):
        acc = results[b * ng]["yT"].astype(np.float32)
        for g in range(1, ng):
            acc = acc + results[b * ng + g]["yT"].astype(np.float32)
        y[b] = acc.T + extra[None, :]
    return y


def kernel(x, wq, bq, wk, bk, wv, bv, wo, bo, mask, _trace=False):
    from concourse.bass_utils import run_bass_kernel_spmd

    causal = bool(np.asarray(mask).item())
    nc = get_nc(causal)
    in_maps = make_in_maps(x, wq, bq, wk, bk, wv, bv, wo, bo)
    res = run_bass_kernel_spmd(nc, in_maps, list(range(NCORES)), trace=_trace)
    y = combine_outputs(res.results, wo, bv, bo)
    if _trace:
        return y, res
    return y


# revision 7
# speedup vs baseline: 1.0536x; 1.0071x over previous
"""Trainium2 Bass kernel for 12-head causal MHA (B=2, S=2048, D=768), bf16 compute.

Sharding: 8 cores = (batch b in {0,1}) x (head-group hg in {0..3}, 3 heads each).

Per-core structure (per 512-column sequence block j):
  - v projection (x-stationary, natural [keys, vdim] layout, N=192)
  - q/k projections, w-stationary, in 4 groups (q01/k01 at 128 rows, q2/k2
    at 64 rows so each head's scores operands share a partition base);
    k bias is mathematically irrelevant under softmax and skipped, q bias
    applied via per-partition tensor_scalar on the PSUM->SBUF copy
  - scores [keys, q] per 128-key tile; full tiles exp'd in [128,1024] pairs,
    diagonal tiles exp'd at exact causal width then masked by a constant
    [128,512] 0/1 triangle (same relative pattern for every diagonal tile)
  - attn@V transposed: out[q, 65] = et^T @ v65 (65 = 64 vdims + ones col for
    the softmax denominator) -- free size 65 instead of 512 halves PE cost
  - normalize per-partition (query) via reciprocal of col 64 + broadcast mul
  - DMA-transpose (XBAR) ot [q, hd] -> otT [hd, q] SBUF->SBUF, heads 0,1
    packed on partitions 0..127 so the output projection contracts 192 dims
    in 2 K-groups
  - y^T partial = wo^T @ otT accumulated over 2 K-groups, copied to bf16 on
    DVE (GPSIMD cannot read PSUM on real HW), DMA'd out

Emission is globally software-pipelined with virtual PE/Act clocks: score+
exp units "feed" the Activation engine while attn@V chains, projections of
neighboring blocks, and the previous block's output projection drain as PE
filler whenever Act has backlog. The last block processes head 2 per
query-subtile so its transposes and output projection pipeline with the
remaining chains.

Host sums the 4 head-group partials per batch (fp32), transposes, and adds
bo + wo @ bv (bv is folded out of the device kernel).
"""

import math
from contextlib import ExitStack

import numpy as np

import concourse.bacc as bacc
import concourse.bass as bass
import concourse.mybir as mybir
import concourse.tile as tile

FP32 = mybir.dt.float32
BF16 = mybir.dt.bfloat16

B = 2
S = 2048
D = 768
NH = 12
DK = 64
NCORES = 8
HG = 3  # heads per core
HD = HG * DK  # 192
KC = D // 128  # 6 contraction chunks
SB = 512  # sequence block
NJ = S // SB  # 4
NT = S // 128  # 16 key tiles
SCALE = 1.0 / math.sqrt(DK)
EXP = mybir.ActivationFunctionType.Exp


def build_nc(causal: bool):
    nc = bacc.Bacc(trn_type="TRN2", target_bir_lowering=False, debug=False)

    x6_d = nc.declare_dram_parameter("x6", [128, KC, S], BF16, isOutput=False)
    wqk_d = nc.declare_dram_parameter("wqk", [128, KC, 3 * 128], BF16, isOutput=False)
    # wqk groups: g0 = wq heads 0,1 | g1 = wk heads 0,1 | g2 = [wq h2 | wk h2]
    wv6_d = nc.declare_dram_parameter("wv6", [128, KC, HD], BF16, isOutput=False)
    wo0_d = nc.declare_dram_parameter("wo0", [128, D], BF16, isOutput=False)
    wo1_d = nc.declare_dram_parameter("wo1", [64, D], BF16, isOutput=False)
    bqg_d = nc.declare_dram_parameter("bqg", [128, 2], FP32, isOutput=False)
    cm4_d = nc.declare_dram_parameter("cm4", [128, 1280], BF16, isOutput=False)
    yT_d = nc.declare_dram_parameter("yT", [D, S], BF16, isOutput=True)

    with tile.TileContext(nc) as tc, ExitStack() as ctx:
        consts = ctx.enter_context(tc.tile_pool(name="consts", bufs=1))

        x6_sb = consts.tile([128, KC, S], BF16)
        wqk_sb = consts.tile([128, KC, 3 * 128], BF16)
        wv6_sb = consts.tile([128, KC, HD], BF16)
        wo0_sb = consts.tile([128, D], BF16)
        wo1_sb = consts.tile([64, D], BF16)
        bqg_sb = consts.tile([128, 2], FP32)
        cm4_sb = consts.tile([128, 1280], BF16)
        qT01_sb = consts.tile([128, S], BF16)  # q heads 0,1
        kT01_sb = consts.tile([128, S], BF16)  # k heads 0,1
        qk2_sb = consts.tile([128, S], BF16)  # rows 0:64 q h2, 64:128 k h2
        kT2_sb = consts.tile([64, S], BF16)  # k head 2 (partition-hopped)
        v65_sb = consts.tile([128, NT, HG, 65], BF16)

        # ---- input DMAs: v weights (SWDGE path, parallel with HWDGE) + x
        # tile 0 first so compute starts early
        nc.gpsimd.dma_start(out=wv6_sb, in_=wv6_d.ap())
        nc.sync.dma_start(out=x6_sb[:, :, 0:128], in_=x6_d.ap()[:, :, 0:128])
        nc.sync.dma_start(out=x6_sb[:, :, 128:SB], in_=x6_d.ap()[:, :, 128:SB])
        nc.sync.dma_start(out=wqk_sb[:, :, 0:128], in_=wqk_d.ap()[:, :, 0:128])
        nc.sync.dma_start(out=wqk_sb[:, :, 128:256], in_=wqk_d.ap()[:, :, 128:256])
        nc.sync.dma_start(out=wqk_sb[:, :, 256:384], in_=wqk_d.ap()[:, :, 256:384])
        nc.scalar.dma_start(out=bqg_sb, in_=bqg_d.ap())
        if causal:
            nc.scalar.dma_start(out=cm4_sb, in_=cm4_d.ap())
        nc.sync.dma_start(
            out=x6_sb[:, :, SB : 2 * SB], in_=x6_d.ap()[:, :, SB : 2 * SB]
        )
        nc.scalar.dma_start(out=wo0_sb, in_=wo0_d.ap())
        nc.scalar.dma_start(out=wo1_sb, in_=wo1_d.ap())
        for j in range(2, NJ):
            eng = nc.sync if j < 3 else nc.scalar
            eng.dma_start(
                out=x6_sb[:, :, j * SB : (j + 1) * SB],
                in_=x6_d.ap()[:, :, j * SB : (j + 1) * SB],
            )

        # ones column for the softmax denominator
        nc.vector.memset(v65_sb[:, :, :, 64:65], 1.0)

        sp_pool = ctx.enter_context(tc.tile_pool(name="sp", bufs=2, space="PSUM"))
        pj_pool = ctx.enter_context(tc.tile_pool(name="pj", bufs=2, space="PSUM"))
        oq_pool = ctx.enter_context(tc.tile_pool(name="oq", bufs=2, space="PSUM"))
        et_pool = ctx.enter_context(tc.tile_pool(name="et", bufs=5))
        ot_pool = ctx.enter_context(tc.tile_pool(name="ot", bufs=3))
        rc_pool = ctx.enter_context(tc.tile_pool(name="rc", bufs=2))
        oT_pool = ctx.enter_context(tc.tile_pool(name="oT", bufs=3))
        yt_pool = ctx.enter_context(tc.tile_pool(name="yt", bufs=6))

        def q_ap(h, j):  # [64, SB] moving operand for scores
            src, base = (
                (qT01_sb, 0) if h == 0 else (qT01_sb, 64) if h == 1 else (qk2_sb, 0)
            )
            return src[base : base + 64, j * SB : (j + 1) * SB]

        def k_ap(h, t):  # [64, 128] stationary operand for scores
            src, base = (
                (kT01_sb, 0) if h == 0 else (kT01_sb, 64) if h == 1 else (kT2_sb, 0)
            )
            return src[base : base + 64, t * 128 : (t + 1) * 128]

        def projv_units(j):
            # v projection: x-stationary, per key tile, N=192
            for st in range(4 * j, 4 * (j + 1)):
                def unit(st=st):
                    vp = pj_pool.tile([128, SB], FP32, name="pj")
                    for c in range(KC):
                        nc.tensor.matmul(
                            vp[:, 0:HD],
                            lhsT=x6_sb[:, c, st * 128 : (st + 1) * 128],
                            rhs=wv6_sb[:, c, :],
                            start=(c == 0),
                            stop=(c == KC - 1),
                        )
                    nc.vector.tensor_copy(
                        v65_sb[:, st, :, 0:64],
                        vp[:, 0:HD].rearrange("p (h d) -> p h d", h=HG),
                    )
                yield unit

        def projqk_units(j, part=None):
            # q/k projections, w-stationary: two 128-row groups (q01, k01)
            # and two 64-row groups (q2, k2) so scores operands share a
            # partition base per head. part "a" = heads 0,1; "b" = head 2.
            jsp = slice(j * SB, (j + 1) * SB)
            groups = (
                (0, qT01_sb, bqg_sb[:, 0:1]),
                (1, kT01_sb, None),
                (2, qk2_sb, bqg_sb[:, 1:2]),
            )
            if part == "a":
                groups = groups[0:2]
            elif part == "b":
                groups = groups[2:3]
            for g, dst, bias in groups:
                def unit(g=g, dst=dst, bias=bias):
                    pp = pj_pool.tile([128, SB], FP32, name="pj")
                    for c in range(KC):
                        nc.tensor.matmul(
                            pp,
                            lhsT=wqk_sb[:, c, g * 128 : (g + 1) * 128],
                            rhs=x6_sb[:, c, jsp],
                            start=(c == 0),
                            stop=(c == KC - 1),
                        )
                    if g == 0:
                        # Act-engine bias-copy: runs in the exp-starved
                        # window at block boundaries, in parallel with the
                        # DVE copy of k01, unblocking the next scores
                        nc.scalar.add(dst[:, jsp], pp, bias)
                    elif bias is not None:
                        nc.vector.tensor_scalar_add(dst[:, jsp], pp, bias)
                    else:
                        nc.vector.tensor_copy(dst[:, jsp], pp)
                    if g == 2:
                        # k2 lives in rows 64:128 of qk2; hop to partitions
                        # 0:64 so scores h2 operands share a partition base
                        nc.gpsimd.dma_start(
                            out=kT2_sb[:, jsp], in_=dst[64:128, jsp]
                        )
                yield unit

        def scores_full_units(j, h, et_t):
            """Full (off-diagonal) score tiles of one head, exp'd in pairs."""
            nfull = 4 * j if causal else NT
            for t0 in range(0, nfull, 2):
                def full_pair(t0=t0):
                    spf = sp_pool.tile([128, 2 * SB], FP32, name="sp")
                    for u in range(2):
                        nc.tensor.matmul(
                            spf[:, u * SB : (u + 1) * SB],
                            lhsT=k_ap(h, t0 + u),
                            rhs=q_ap(h, j),
                            start=True,
                            stop=True,
                        )
                    nc.scalar.activation(
                        et_t[:, t0 : t0 + 2, :], spf, EXP, scale=SCALE
                    )
                yield full_pair

        # packed-diagonal layout: tile 4j+0 (full width) stays in its
        # normal et slot; tiles 4j+1..3 (widths 384/256/128) are packed
        # back-to-back into the 768 columns starting at et slot 4j+1, so
        # one exp + one mask-mul covers all three.
        DSEG = (0, 0, 384, 640)  # packed base offset of diag tile u (u>=1)

        def av_lhsT(et_t, j, t, qs):
            u = t - 4 * j
            if not causal or u < 1:
                return et_t[:, t, qs * 128 : (qs + 1) * 128]
            flat = et_t[:, :, :].rearrange("p a b -> p (a b)")
            base = (4 * j + 1) * SB + DSEG[u] + qs * 128 - 128 * u
            return flat[:, base : base + 128]

        def scores_diag_units(j, h, et_t):
            """Diagonal score tiles at exact causal width, then 0/1 mask."""
            def diag0():
                t = 4 * j
                spd = sp_pool.tile([128, 2 * SB], FP32, name="sp")
                nc.tensor.matmul(
                    spd[:, 0:SB],
                    lhsT=k_ap(h, t),
                    rhs=q_ap(h, j),
                    start=True,
                    stop=True,
                )
                nc.scalar.activation(et_t[:, t, :], spd[:, 0:SB], EXP, scale=SCALE)
                nc.vector.tensor_mul(
                    et_t[:, t, :], et_t[:, t, :], cm4_sb[:, 0:SB]
                )
            yield diag0

            def diag123():
                # PSUM segments offset so no matmul write crosses a 2KB
                # bank boundary: u1 at [128:512], u2 [512:768], u3 [768:896];
                # the exp reads the contiguous [128:896] span and writes the
                # gapless packed et region
                spd = sp_pool.tile([128, 2 * SB], FP32, name="sp")
                for u in range(1, 4):
                    t = 4 * j + u
                    off = 128 * u
                    n = SB - off
                    pseg = (0, 128, 512, 768)[u]
                    nc.tensor.matmul(
                        spd[:, pseg : pseg + n],
                        lhsT=k_ap(h, t),
                        rhs=q_ap(h, j)[:, off:SB],
                        start=True,
                        stop=True,
                    )
                flat = et_t[:, :, :].rearrange("p a b -> p (a b)")
                dst = flat[:, (4 * j + 1) * SB : (4 * j + 1) * SB + 768]
                nc.scalar.activation(dst, spd[:, 128:896], EXP, scale=SCALE)
                nc.vector.tensor_mul(dst, dst, cm4_sb[:, SB : SB + 768])
            yield diag123

        def attnv_units(j, h, et_t, rc_t, ot4):
            """attn@V chains + normalize for one head."""
            tend = 4 * (j + 1) if causal else NT
            oq_t = oq_pool.tile([128, 4, 128], FP32)
            for qs in range(4):
                def chain(qs=qs):
                    tq = (4 * j + qs + 1) if causal else tend
                    for t in range(tq):
                        nc.tensor.matmul(
                            oq_t[:, qs, 0:65],
                            lhsT=av_lhsT(et_t, j, t, qs),
                            rhs=v65_sb[:, t, h, :],
                            start=(t == 0),
                            stop=(t == tq - 1),
                        )
                yield chain

            def normalize():
                # normalize immediately so the oq buffer frees early:
                # ot4[q, qs, h, :] = oq[q, qs, 0:64] / oq[q, qs, 64]
                nc.vector.reciprocal(rc_t[:, h * 4 : (h + 1) * 4], oq_t[:, :, 64])
                nc.vector.tensor_mul(
                    ot4[:, :, h, :],
                    oq_t[:, :, 0:64],
                    rc_t[:, h * 4 : (h + 1) * 4].unsqueeze(-1).to_broadcast(
                        (128, 4, 64)
                    ),
                )
            yield normalize

        def yproj_units(j, oT01, oT2, copy_engines):
            jsp = slice(j * SB, (j + 1) * SB)
            # output projection: 2 contraction groups (128 + 64)
            for dt in range(KC):
                def unit(dt=dt):
                    yp = pj_pool.tile([128, SB], FP32, name="pj")
                    nc.tensor.matmul(
                        yp,
                        lhsT=wo0_sb[:, dt * 128 : (dt + 1) * 128],
                        rhs=oT01,
                        start=True,
                        stop=False,
                    )
                    nc.tensor.matmul(
                        yp,
                        lhsT=wo1_sb[:, dt * 128 : (dt + 1) * 128],
                        rhs=oT2[0:64, :],
                        start=False,
                        stop=True,
                    )
                    yt = yt_pool.tile([128, SB], BF16)
                    eng = copy_engines[dt % len(copy_engines)]
                    if eng is nc.scalar:
                        eng.copy(yt, yp)
                    else:
                        eng.tensor_copy(yt, yp)
                    nc.sync.dma_start(
                        out=yT_d.ap()[dt * 128 : (dt + 1) * 128, jsp], in_=yt
                    )
                yield unit

        def interleave(feeder, filler):
            """Emit feeder units (which keep Act busy) with filler PE units
            spread evenly between them; leftover fillers go at the end."""
            feeder = list(feeder)
            filler = list(filler)
            nf = len(feeder)
            emitted = 0
            for i, f in enumerate(feeder):
                f()
                want = (i + 1) * len(filler) // nf if nf else len(filler)
                while emitted < want:
                    filler[emitted]()
                    emitted += 1
            while emitted < len(filler):
                filler[emitted]()
                emitted += 1

        def transposes01(ot4):
            # XBAR transposes for heads 0,1 (packed on partitions 0..127)
            oT01 = oT_pool.tile([128, SB], BF16, name="oT01")
            for qs in range(4):
                nc.sync.dma_start_transpose(
                    out=oT01[:, qs * 128 : (qs + 1) * 128], in_=ot4[:, qs, 0:2, :]
                )
            return oT01

        def transposes2(ot4):
            # XBAR transpose for head 2 (+pad rows, never consumed)
            oT2 = oT_pool.tile([128, SB], BF16, name="oT2")
            for qs in range(4):
                nc.sync.dma_start_transpose(
                    out=oT2[:, qs * 128 : (qs + 1) * 128], in_=ot4[:, qs, 2:4, :]
                )
            return oT2

        def run(units):
            for u in units:
                u()

        # Software-pipelined global schedule driven by virtual PE/Act
        # clocks: score+exp units are "feeders" (they load both engines),
        # everything else is PE-only "filler". A feeder is emitted when the
        # Act backlog is small (sp pool depth limits PE run-ahead anyway);
        # fillers drain while Act chews. Fillers carry across phases.
        PE_CYC = 0.4167

        def fp_cost(_):  # full pair: 2 scores + [128,1024] exp
            return 2 * SB * PE_CYC, 1024 * 0.833 + 185

        def dg_cost(u):  # diag tile u: score + exp + mask
            n = SB - 128 * u
            return n * PE_CYC, n * 0.833 + 185

        class tposes:
            oT01 = None
            oT2 = None

        if causal:
            clocks = {"pe": 0.0, "act": 0.0}
            fillers = []

            def emit_feeder(u, pe, act):
                u()
                clocks["pe"] += pe
                clocks["act"] = max(clocks["act"], clocks["pe"]) + act

            def emit_filler():
                pe, u = fillers.pop(0)
                u()
                clocks["pe"] += pe

            def phase(feeders):
                for u, pe, act in feeders:
                    # drain fillers while Act has >1.4us of backlog
                    while fillers and clocks["act"] - clocks["pe"] > 1400:
                        emit_filler()
                    emit_feeder(u, pe, act)

            def add_fillers(units, pe_each):
                fillers.extend((pe_each, u) for u in units)

            pv0 = list(projv_units(0))
            run(pv0)
            for u in projqk_units(0, "a"):
                u()
                clocks["pe"] += 1280
            ets = {}

            def et(j, h):
                if (j, h) not in ets:
                    ets[(j, h)] = et_pool.tile([128, NT, SB], BF16, name="et")
                return ets[(j, h)]

            def feed_full(j, h):
                return [(u, *fp_cost(0)) for u in scores_full_units(j, h, et(j, h))]

            def feed_diag(j, h):
                costs = [(SB * 0.4167, 612.0), (768 * 0.4167, 825.0)]
                return [
                    (u, *costs[i])
                    for i, u in enumerate(scores_diag_units(j, h, et(j, h)))
                ]

            pqkb_done = {}

            def mark(j):
                def m():
                    pqkb_done[j] = True
                return m

            for j in range(NJ):
                rc_t = rc_pool.tile([128, HG * 4], FP32)
                ot4 = ot_pool.tile([128, 4, 4, 64], BF16)
                nc.vector.memset(ot4[:, :, 3, :], 0.0)
                if j == 0:
                    add_fillers(projqk_units(0, "b"), 1280)
                    add_fillers([mark(0)], 0)
                    phase(feed_full(0, 0) + feed_diag(0, 0))
                else:
                    phase(feed_diag(j, 0))
                av0 = list(attnv_units(j, 0, ets.pop((j, 0)), rc_t, ot4))
                add_fillers(av0, 27 * (4 * j + 3))
                phase(feed_full(j, 1) + feed_diag(j, 1))
                av1 = list(attnv_units(j, 1, ets.pop((j, 1)), rc_t, ot4))
                add_fillers(av1, 27 * (4 * j + 3))
                add_fillers(
                    [lambda ot4=ot4: setattr(tposes, "oT01", transposes01(ot4))], 0
                )
                # head-2 q/k of this block must be in SBUF before its scores
                while fillers and not pqkb_done.get(j, False):
                    emit_filler()
                phase(feed_full(j, 2) + feed_diag(j, 2))
                if j + 1 < NJ:
                    av2 = list(attnv_units(j, 2, ets.pop((j, 2)), rc_t, ot4))
                    # q01/k01 of the next block precede its scores; Act still
                    # has the h2-scores backlog to chew while PE projects
                    while fillers:
                        emit_filler()
                    for u in projqk_units(j + 1, "a"):
                        u()
                        clocks["pe"] += 1280
                    add_fillers(av2, 27 * (4 * j + 3))
                    add_fillers(
                        [lambda ot4=ot4: setattr(tposes, "oT2", transposes2(ot4))], 0
                    )
                    phase(feed_full(j + 1, 0))
                    add_fillers(projqk_units(j + 1, "b"), 1280)
                    add_fillers([mark(j + 1)], 0)
                    add_fillers(projv_units(j + 1), 480)
                    while fillers and tposes.oT2 is None:
                        emit_filler()
                    oT01, oT2 = tposes.oT01, tposes.oT2
                    tposes.oT01 = tposes.oT2 = None
                    add_fillers(
                        yproj_units(j, oT01, oT2, (nc.vector, nc.vector, nc.vector)),
                        426,
                    )
                else:
                    # ---- last block: process head 2 per query-subtile so
                    # the XBAR transposes and the output projection pipeline
                    # with the remaining attn@V chains
                    et_t = ets.pop((j, 2))
                    oq_t = oq_pool.tile([128, 4, 128], FP32)
                    oT2 = oT_pool.tile([128, SB], BF16, name="oT2")
                    for qs in range(4):
                        tq = 4 * j + qs + 1
                        for t in range(tq):
                            nc.tensor.matmul(
                                oq_t[:, qs, 0:65],
                                lhsT=av_lhsT(et_t, j, t, qs),
                                rhs=v65_sb[:, t, 2, :],
                                start=(t == 0),
                                stop=(t == tq - 1),
                            )
                        nc.vector.reciprocal(
                            rc_t[:, 8 + qs : 9 + qs], oq_t[:, qs, 64:65]
                        )
                        nc.vector.tensor_mul(
                            ot4[:, qs, 2, :],
                            oq_t[:, qs, 0:64],
                            rc_t[:, 8 + qs : 9 + qs].to_broadcast((128, 64)),
                        )
                        eng = nc.scalar if qs % 2 else nc.sync
                        eng.dma_start_transpose(
                            out=oT2[:, qs * 128 : (qs + 1) * 128],
                            in_=ot4[:, qs, 2:4, :],
                        )
                        if qs == 1:
                            while fillers:
                                emit_filler()
                    oT01 = tposes.oT01
                    tposes.oT01 = None
                    run(yproj_units(j, oT01, oT2, (nc.vector, nc.scalar, nc.vector)))
            while fillers:
                emit_filler()
        else:
            run(projqk_units(0))
            for j in range(NJ):
                run(projv_units(j))
                if j + 1 < NJ:
                    run(projqk_units(j + 1))
            yprev = []
            for j in range(NJ):
                rc_t = rc_pool.tile([128, HG * 4], FP32)
                ot4 = ot_pool.tile([128, 4, 4, 64], BF16)
                nc.vector.memset(ot4[:, :, 3, :], 0.0)
                ets = [
                    et_pool.tile([128, NT, SB], BF16, name="et") for _ in range(HG)
                ]
                interleave(scores_full_units(j, 0, ets[0]), yprev)
                interleave(
                    scores_full_units(j, 1, ets[1]),
                    attnv_units(j, 0, ets[0], rc_t, ot4),
                )
                interleave(
                    scores_full_units(j, 2, ets[2]),
                    attnv_units(j, 1, ets[1], rc_t, ot4),
                )
                run(attnv_units(j, 2, ets[2], rc_t, ot4))
                oT01 = transposes01(ot4)
                oT2 = transposes2(ot4)
                engines = (nc.vector, nc.vector, nc.vector)
                yprev = list(yproj_units(j, oT01, oT2, engines))
                for u in yprev:
                    u()
                yprev = []

    nc.finalize()
    return nc


_NC_CACHE: dict[bool, object] = {}


def get_nc(causal: bool):
    if causal not in _NC_CACHE:
        _NC_CACHE[causal] = build_nc(causal)
    return _NC_CACHE[causal]


def _bf16(a):
    import ml_dtypes

    return np.asarray(a, np.float32).astype(ml_dtypes.bfloat16)


def _chunked(wT):
    """[768, N] -> [128, 6, N] with chunk c = rows 128c..128c+127."""
    n = wT.shape[1]
    return np.ascontiguousarray(wT.reshape(KC, 128, n).transpose(1, 0, 2))


def _make_cm4():
    # packed relative triangle masks [tri512 | tri384 | tri256 | tri128]:
    # tri(n)[p, c] = 1.0 iff c >= p; diagonal tile u uses tri(512 - 128u)
    p = np.arange(128)[:, None]
    segs = []
    for n in (512, 384, 256, 128):
        c = np.arange(n)[None, :]
        segs.append((c >= p).astype(np.float32))
    return np.concatenate(segs, axis=1)


def make_in_maps(x, wq, bq, wk, bk, wv, bv, wo, bo):
    f32 = np.float32
    x = np.asarray(x, f32)
    cm4 = _bf16(_make_cm4())
    in_maps = []
    for core in range(NCORES):
        b, hg = divmod(core, NH // HG)
        hs = slice(hg * HD, (hg + 1) * HD)
        wqT = np.asarray(wq, f32)[hs, :].T  # [768, 192]
        wkT = np.asarray(wk, f32)[hs, :].T
        wqkT = np.concatenate(
            [wqT[:, 0:128], wkT[:, 0:128], wqT[:, 128:192], wkT[:, 128:192]], axis=1
        )
        bqc = np.asarray(bq, f32)[hs]
        bqg = np.zeros((128, 2), f32)
        bqg[:, 0] = bqc[0:128]
        bqg[0:64, 1] = bqc[128:192]
        woT = np.asarray(wo, f32)[:, hs].T  # [192, 768]
        in_maps.append(
            {
                "x6": _bf16(_chunked(np.ascontiguousarray(x[b].T))),
                "wqk": _bf16(_chunked(wqkT)),
                "wv6": _bf16(_chunked(np.asarray(wv, f32)[hs, :].T)),
                "wo0": _bf16(woT[0:128, :]),
                "wo1": _bf16(woT[128:192, :]),
                "bqg": bqg,
                "cm4": cm4,
            }
        )
    return in_maps


def combine_outputs(results, wo, bv, bo):
    y = np.empty((B, S, D), np.float32)
    ng = NH // HG
    extra = (np.asarray(wo, np.float32) @ np.asarray(bv, np.float32)) + np.asarray(
        bo, np.float32
    )
    for b in range(B):
        acc = results[b * ng]["yT"].astype(np.float32)
        for g in range(1, ng):
            acc = acc + results[b * ng + g]["yT"].astype(np.float32)
        y[b] = acc.T + extra[None, :]
    return y


def kernel(x, wq, bq, wk, bk, wv, bv, wo, bo, mask, _trace=False):
    from concourse.bass_utils import run_bass_kernel_spmd

    causal = bool(np.asarray(mask).item())
    nc = get_nc(causal)
    in_maps = make_in_maps(x, wq, bq, wk, bk, wv, bv, wo, bo)
    res = run_bass_kernel_spmd(nc, in_maps, list(range(NCORES)), trace=_trace)
    y = combine_outputs(res.results, wo, bv, bo)
    if _trace:
        return y, res
    return y


# revision 8
# speedup vs baseline: 1.1039x; 1.0478x over previous
"""Trainium2 Bass kernel for 12-head causal MHA (B=2, S=2048, D=768), bf16 compute.

Sharding: 8 cores = (batch b in {0,1}) x (head-group hg in {0..3}, 3 heads each).

Per-core structure (per 512-column sequence block j):
  - v projection (x-stationary, natural [keys, vdim] layout, N=192)
  - q/k projections, w-stationary, in 4 groups (q01/k01 at 128 rows, q2/k2
    at 64 rows so each head's scores operands share a partition base);
    k bias is mathematically irrelevant under softmax and skipped, q bias
    applied via per-partition tensor_scalar on the PSUM->SBUF copy
  - scores [keys, q] per 128-key tile; full tiles exp'd in [128,1024] pairs,
    diagonal tiles exp'd at exact causal width then masked by a constant
    [128,512] 0/1 triangle (same relative pattern for every diagonal tile)
  - attn@V transposed: out[q, 65] = et^T @ v65 (65 = 64 vdims + ones col for
    the softmax denominator) -- free size 65 instead of 512 halves PE cost
  - normalize per-partition (query) via reciprocal of col 64 + broadcast mul
  - DMA-transpose (XBAR) ot [q, hd] -> otT [hd, q] SBUF->SBUF, heads 0,1
    packed on partitions 0..127 so the output projection contracts 192 dims
    in 2 K-groups
  - y^T partial = wo^T @ otT accumulated over 2 K-groups, copied to bf16 on
    DVE (GPSIMD cannot read PSUM on real HW), DMA'd out

Emission is globally software-pipelined with virtual PE/Act clocks: score+
exp units "feed" the Activation engine while attn@V chains, projections of
neighboring blocks, and the previous block's output projection drain as PE
filler whenever Act has backlog. The last block processes head 2 per
query-subtile so its transposes and output projection pipeline with the
remaining chains.

Host sums the 4 head-group partials per batch (fp32), transposes, and adds
bo + wo @ bv (bv is folded out of the device kernel).
"""

import math
from contextlib import ExitStack

import numpy as np

import concourse.bacc as bacc
import concourse.bass as bass
import concourse.mybir as mybir
import concourse.tile as tile

FP32 = mybir.dt.float32
BF16 = mybir.dt.bfloat16

B = 2
S = 2048
D = 768
NH = 12
DK = 64
NCORES = 8
HG = 3  # heads per core
HD = HG * DK  # 192
KC = D // 128  # 6 contraction chunks
SB = 512  # sequence block
NJ = S // SB  # 4
NT = S // 128  # 16 key tiles
SCALE = 1.0 / math.sqrt(DK)
EXP = mybir.ActivationFunctionType.Exp


def build_nc(causal: bool):
    nc = bacc.Bacc(trn_type="TRN2", target_bir_lowering=False, debug=False)

    x6_d = nc.declare_dram_parameter("x6", [128, KC, S], BF16, isOutput=False)
    wqk_d = nc.declare_dram_parameter("wqk", [128, KC, 3 * 128], BF16, isOutput=False)
    # wqk groups: g0 = wq heads 0,1 | g1 = wk heads 0,1 | g2 = [wq h2 | wk h2]
    wv6_d = nc.declare_dram_parameter("wv6", [128, KC, HD], BF16, isOutput=False)
    wo0_d = nc.declare_dram_parameter("wo0", [128, D], BF16, isOutput=False)
    wo1_d = nc.declare_dram_parameter("wo1", [64, D], BF16, isOutput=False)
    bqg_d = nc.declare_dram_parameter("bqg", [128, 2], FP32, isOutput=False)
    cm4_d = nc.declare_dram_parameter("cm4", [128, 1280], BF16, isOutput=False)
    yT_d = nc.declare_dram_parameter("yT", [D, S], BF16, isOutput=True)

    with tile.TileContext(nc) as tc, ExitStack() as ctx:
        consts = ctx.enter_context(tc.tile_pool(name="consts", bufs=1))

        x6_sb = consts.tile([128, KC, S], BF16)
        wqk_sb = consts.tile([128, KC, 3 * 128], BF16)
        wv6_sb = consts.tile([128, KC, HD], BF16)
        wo0_sb = consts.tile([128, D], BF16)
        wo1_sb = consts.tile([64, D], BF16)
        bqg_sb = consts.tile([128, 2], FP32)
        cm4_sb = consts.tile([128, 1280], BF16)
        qT01_sb = consts.tile([128, S], BF16)  # q heads 0,1
        kT01_sb = consts.tile([128, S], BF16)  # k heads 0,1
        qk2_sb = consts.tile([128, S], BF16)  # rows 0:64 q h2, 64:128 k h2
        kT2_sb = consts.tile([64, S], BF16)  # k head 2 (partition-hopped)
        v65_sb = consts.tile([128, NT, HG, 65], BF16)

        # ---- input DMAs: v weights (SWDGE path, parallel with HWDGE) + x
        # tile 0 first so compute starts early
        nc.gpsimd.dma_start(out=wv6_sb, in_=wv6_d.ap())
        nc.sync.dma_start(out=x6_sb[:, :, 0:128], in_=x6_d.ap()[:, :, 0:128])
        nc.sync.dma_start(out=x6_sb[:, :, 128:SB], in_=x6_d.ap()[:, :, 128:SB])
        nc.sync.dma_start(out=wqk_sb[:, :, 0:128], in_=wqk_d.ap()[:, :, 0:128])
        nc.sync.dma_start(out=wqk_sb[:, :, 128:256], in_=wqk_d.ap()[:, :, 128:256])
        nc.sync.dma_start(out=wqk_sb[:, :, 256:384], in_=wqk_d.ap()[:, :, 256:384])
        nc.scalar.dma_start(out=bqg_sb, in_=bqg_d.ap())
        if causal:
            nc.scalar.dma_start(out=cm4_sb, in_=cm4_d.ap())
        nc.sync.dma_start(
            out=x6_sb[:, :, SB : 2 * SB], in_=x6_d.ap()[:, :, SB : 2 * SB]
        )
        nc.scalar.dma_start(out=wo0_sb, in_=wo0_d.ap())
        nc.scalar.dma_start(out=wo1_sb, in_=wo1_d.ap())
        for j in range(2, NJ):
            eng = nc.sync if j < 3 else nc.scalar
            eng.dma_start(
                out=x6_sb[:, :, j * SB : (j + 1) * SB],
                in_=x6_d.ap()[:, :, j * SB : (j + 1) * SB],
            )

        # ones column for the softmax denominator
        nc.vector.memset(v65_sb[:, :, :, 64:65], 1.0)

        sp_pool = ctx.enter_context(tc.tile_pool(name="sp", bufs=2, space="PSUM"))
        pj_pool = ctx.enter_context(tc.tile_pool(name="pj", bufs=2, space="PSUM"))
        oq_pool = ctx.enter_context(tc.tile_pool(name="oq", bufs=2, space="PSUM"))
        et_pool = ctx.enter_context(tc.tile_pool(name="et", bufs=5))
        ot_pool = ctx.enter_context(tc.tile_pool(name="ot", bufs=3))
        rc_pool = ctx.enter_context(tc.tile_pool(name="rc", bufs=2))
        oT_pool = ctx.enter_context(tc.tile_pool(name="oT", bufs=3))
        yt_pool = ctx.enter_context(tc.tile_pool(name="yt", bufs=6))

        def q_ap(h, j):  # [64, SB] moving operand for scores
            src, base = (
                (qT01_sb, 0) if h == 0 else (qT01_sb, 64) if h == 1 else (qk2_sb, 0)
            )
            return src[base : base + 64, j * SB : (j + 1) * SB]

        def k_ap(h, t):  # [64, 128] stationary operand for scores
            src, base = (
                (kT01_sb, 0) if h == 0 else (kT01_sb, 64) if h == 1 else (kT2_sb, 0)
            )
            return src[base : base + 64, t * 128 : (t + 1) * 128]

        def projv_units(j):
            # v projection: x-stationary, per key tile, N=192
            for st in range(4 * j, 4 * (j + 1)):
                def unit(st=st):
                    vp = pj_pool.tile([128, SB], FP32, name="pj")
                    for c in range(KC):
                        nc.tensor.matmul(
                            vp[:, 0:HD],
                            lhsT=x6_sb[:, c, st * 128 : (st + 1) * 128],
                            rhs=wv6_sb[:, c, :],
                            start=(c == 0),
                            stop=(c == KC - 1),
                        )
                    nc.vector.tensor_copy(
                        v65_sb[:, st, :, 0:64],
                        vp[:, 0:HD].rearrange("p (h d) -> p h d", h=HG),
                    )
                yield unit

        def projqk_units(j, part=None):
            # q/k projections, w-stationary: two 128-row groups (q01, k01)
            # and two 64-row groups (q2, k2) so scores operands share a
            # partition base per head. part "a" = heads 0,1; "b" = head 2.
            jsp = slice(j * SB, (j + 1) * SB)
            groups = (
                (0, qT01_sb, bqg_sb[:, 0:1]),
                (1, kT01_sb, None),
                (2, qk2_sb, bqg_sb[:, 1:2]),
            )
            if part == "a":
                groups = groups[0:2]
            elif part == "b":
                groups = groups[2:3]
            for g, dst, bias in groups:
                def unit(g=g, dst=dst, bias=bias):
                    pp = pj_pool.tile([128, SB], FP32, name="pj")
                    for c in range(KC):
                        nc.tensor.matmul(
                            pp,
                            lhsT=wqk_sb[:, c, g * 128 : (g + 1) * 128],
                            rhs=x6_sb[:, c, jsp],
                            start=(c == 0),
                            stop=(c == KC - 1),
                        )
                    if g == 0:
                        # Act-engine bias-copy: runs in the exp-starved
                        # window at block boundaries, in parallel with the
                        # DVE copy of k01, unblocking the next scores
                        nc.scalar.add(dst[:, jsp], pp, bias)
                    elif bias is not None:
                        nc.vector.tensor_scalar_add(dst[:, jsp], pp, bias)
                    else:
                        nc.vector.tensor_copy(dst[:, jsp], pp)
                    if g == 2:
                        # k2 lives in rows 64:128 of qk2; hop to partitions
                        # 0:64 so scores h2 operands share a partition base
                        nc.gpsimd.dma_start(
                            out=kT2_sb[:, jsp], in_=dst[64:128, jsp]
                        )
                yield unit

        def scores_full_units(j, h, et_t):
            """Full (off-diagonal) score tiles of one head, exp'd in pairs."""
            nfull = 4 * j if causal else NT
            for t0 in range(0, nfull, 2):
                def full_pair(t0=t0):
                    spf = sp_pool.tile([128, 2 * SB], FP32, name="sp")
                    for u in range(2):
                        nc.tensor.matmul(
                            spf[:, u * SB : (u + 1) * SB],
                            lhsT=k_ap(h, t0 + u),
                            rhs=q_ap(h, j),
                            start=True,
                            stop=True,
                        )
                    nc.scalar.activation(
                        et_t[:, t0 : t0 + 2, :], spf, EXP, scale=SCALE
                    )
                yield full_pair

        # packed-diagonal layout: tile 4j+0 (full width) stays in its
        # normal et slot; tiles 4j+1..3 (widths 384/256/128) are packed
        # back-to-back into the 768 columns starting at et slot 4j+1, so
        # one exp + one mask-mul covers all three.
        DSEG = (0, 0, 384, 640)  # packed base offset of diag tile u (u>=1)

        def av_lhsT(et_t, j, t, qs):
            u = t - 4 * j
            if not causal or u < 1:
                return et_t[:, t, qs * 128 : (qs + 1) * 128]
            flat = et_t[:, :, :].rearrange("p a b -> p (a b)")
            base = (4 * j + 1) * SB + DSEG[u] + qs * 128 - 128 * u
            return flat[:, base : base + 128]

        def scores_diag_units(j, h, et_t):
            """Diagonal score tiles at exact causal width, then 0/1 mask."""
            def diag0():
                t = 4 * j
                spd = sp_pool.tile([128, 2 * SB], FP32, name="sp")
                nc.tensor.matmul(
                    spd[:, 0:SB],
                    lhsT=k_ap(h, t),
                    rhs=q_ap(h, j),
                    start=True,
                    stop=True,
                )
                nc.scalar.activation(et_t[:, t, :], spd[:, 0:SB], EXP, scale=SCALE)
                nc.vector.tensor_mul(
                    et_t[:, t, :], et_t[:, t, :], cm4_sb[:, 0:SB]
                )
            yield diag0

            def diag123():
                # PSUM segments offset so no matmul write crosses a 2KB
                # bank boundary: u1 at [128:512], u2 [512:768], u3 [768:896];
                # the exp reads the contiguous [128:896] span and writes the
                # gapless packed et region
                spd = sp_pool.tile([128, 2 * SB], FP32, name="sp")
                for u in range(1, 4):
                    t = 4 * j + u
                    off = 128 * u
                    n = SB - off
                    pseg = (0, 128, 512, 768)[u]
                    nc.tensor.matmul(
                        spd[:, pseg : pseg + n],
                        lhsT=k_ap(h, t),
                        rhs=q_ap(h, j)[:, off:SB],
                        start=True,
                        stop=True,
                    )
                flat = et_t[:, :, :].rearrange("p a b -> p (a b)")
                dst = flat[:, (4 * j + 1) * SB : (4 * j + 1) * SB + 768]
                nc.scalar.activation(dst, spd[:, 128:896], EXP, scale=SCALE)
                nc.vector.tensor_mul(dst, dst, cm4_sb[:, SB : SB + 768])
            yield diag123

        def attnv_units(j, h, et_t, rc_t, ot4):
            """attn@V chains + normalize for one head."""
            tend = 4 * (j + 1) if causal else NT
            oq_t = oq_pool.tile([128, 4, 128], FP32)
            for qs in range(4):
                def chain(qs=qs):
                    tq = (4 * j + qs + 1) if causal else tend
                    for t in range(tq):
                        nc.tensor.matmul(
                            oq_t[:, qs, 0:65],
                            lhsT=av_lhsT(et_t, j, t, qs),
                            rhs=v65_sb[:, t, h, :],
                            start=(t == 0),
                            stop=(t == tq - 1),
                        )
                yield chain

            def normalize():
                # normalize immediately so the oq buffer frees early:
                # ot4[q, qs, h, :] = oq[q, qs, 0:64] / oq[q, qs, 64]
                nc.vector.reciprocal(rc_t[:, h * 4 : (h + 1) * 4], oq_t[:, :, 64])
                nc.vector.tensor_mul(
                    ot4[:, :, h, :],
                    oq_t[:, :, 0:64],
                    rc_t[:, h * 4 : (h + 1) * 4].unsqueeze(-1).to_broadcast(
                        (128, 4, 64)
                    ),
                )
            yield normalize

        def yproj_units(j, oT01, oT2, copy_engines):
            jsp = slice(j * SB, (j + 1) * SB)
            # output projection: 2 contraction groups (128 + 64)
            for dt in range(KC):
                def unit(dt=dt):
                    yp = pj_pool.tile([128, SB], FP32, name="pj")
                    nc.tensor.matmul(
                        yp,
                        lhsT=wo0_sb[:, dt * 128 : (dt + 1) * 128],
                        rhs=oT01,
                        start=True,
                        stop=False,
                    )
                    nc.tensor.matmul(
                        yp,
                        lhsT=wo1_sb[:, dt * 128 : (dt + 1) * 128],
                        rhs=oT2[0:64, :],
                        start=False,
                        stop=True,
                    )
                    yt = yt_pool.tile([128, SB], BF16)
                    eng = copy_engines[dt % len(copy_engines)]
                    if eng is nc.scalar:
                        eng.copy(yt, yp)
                    else:
                        eng.tensor_copy(yt, yp)
                    nc.sync.dma_start(
                        out=yT_d.ap()[dt * 128 : (dt + 1) * 128, jsp], in_=yt
                    )
                yield unit

        def interleave(feeder, filler):
            """Emit feeder units (which keep Act busy) with filler PE units
            spread evenly between them; leftover fillers go at the end."""
            feeder = list(feeder)
            filler = list(filler)
            nf = len(feeder)
            emitted = 0
            for i, f in enumerate(feeder):
                f()
                want = (i + 1) * len(filler) // nf if nf else len(filler)
                while emitted < want:
                    filler[emitted]()
                    emitted += 1
            while emitted < len(filler):
                filler[emitted]()
                emitted += 1

        def transposes01(ot4):
            # XBAR transposes for heads 0,1 (packed on partitions 0..127)
            oT01 = oT_pool.tile([128, SB], BF16, name="oT01")
            for qs in range(4):
                nc.sync.dma_start_transpose(
                    out=oT01[:, qs * 128 : (qs + 1) * 128], in_=ot4[:, qs, 0:2, :]
                )
            return oT01

        def transposes2(ot4):
            # XBAR transpose for head 2 (+pad rows, never consumed)
            oT2 = oT_pool.tile([128, SB], BF16, name="oT2")
            for qs in range(4):
                nc.sync.dma_start_transpose(
                    out=oT2[:, qs * 128 : (qs + 1) * 128], in_=ot4[:, qs, 2:4, :]
                )
            return oT2

        def run(units):
            for u in units:
                u()

        # Software-pipelined global schedule driven by virtual PE/Act
        # clocks: score+exp units are "feeders" (they load both engines),
        # everything else is PE-only "filler". A feeder is emitted when the
        # Act backlog is small (sp pool depth limits PE run-ahead anyway);
        # fillers drain while Act chews. Fillers carry across phases.
        PE_CYC = 0.4167

        def fp_cost(_):  # full pair: 2 scores + [128,1024] exp
            return 2 * SB * PE_CYC, 1024 * 0.833 + 185

        def dg_cost(u):  # diag tile u: score + exp + mask
            n = SB - 128 * u
            return n * PE_CYC, n * 0.833 + 185

        class tposes:
            oT01 = None
            oT2 = None

        if causal:
            clocks = {"pe": 0.0, "act": 0.0}
            fillers = []

            def emit_feeder(u, pe, act):
                u()
                clocks["pe"] += pe
                clocks["act"] = max(clocks["act"], clocks["pe"]) + act

            def emit_filler():
                pe, u = fillers.pop(0)
                u()
                clocks["pe"] += pe

            def phase(feeders):
                for u, pe, act in feeders:
                    # drain fillers while Act has >1.4us of backlog
                    while fillers and clocks["act"] - clocks["pe"] > 1400:
                        emit_filler()
                    emit_feeder(u, pe, act)

            def add_fillers(units, pe_each):
                fillers.extend((pe_each, u) for u in units)

            pv0 = list(projv_units(0))
            run(pv0)
            for u in projqk_units(0, "a"):
                u()
                clocks["pe"] += 1280
            ets = {}

            def et(j, h):
                if (j, h) not in ets:
                    ets[(j, h)] = et_pool.tile([128, NT, SB], BF16, name="et")
                return ets[(j, h)]

            def feed_full(j, h):
                return [(u, *fp_cost(0)) for u in scores_full_units(j, h, et(j, h))]

            def feed_diag(j, h):
                costs = [(SB * 0.4167, 612.0), (768 * 0.4167, 825.0)]
                return [
                    (u, *costs[i])
                    for i, u in enumerate(scores_diag_units(j, h, et(j, h)))
                ]

            pqkb_done = {}

            def mark(j):
                def m():
                    pqkb_done[j] = True
                return m

            for j in range(NJ):
                rc_t = rc_pool.tile([128, HG * 4], FP32)
                ot4 = ot_pool.tile([128, 4, 4, 64], BF16)
                nc.vector.memset(ot4[:, :, 3, :], 0.0)
                if j == 0:
                    add_fillers(projqk_units(0, "b"), 1280)
                    add_fillers([mark(0)], 0)
                    phase(feed_full(0, 0) + feed_diag(0, 0))
                else:
                    phase(feed_diag(j, 0))
                av0 = list(attnv_units(j, 0, ets.pop((j, 0)), rc_t, ot4))
                add_fillers(av0, 27 * (4 * j + 3))
                phase(feed_full(j, 1) + feed_diag(j, 1))
                av1 = list(attnv_units(j, 1, ets.pop((j, 1)), rc_t, ot4))
                add_fillers(av1, 27 * (4 * j + 3))
                add_fillers(
                    [lambda ot4=ot4: setattr(tposes, "oT01", transposes01(ot4))], 0
                )
                # head-2 q/k of this block must be in SBUF before its scores
                while fillers and not pqkb_done.get(j, False):
                    emit_filler()
                phase(feed_full(j, 2) + feed_diag(j, 2))
                if j + 1 < NJ:
                    av2 = list(attnv_units(j, 2, ets.pop((j, 2)), rc_t, ot4))
                    # q01/k01 of the next block precede its scores; Act still
                    # has the h2-scores backlog to chew while PE projects
                    while fillers:
                        emit_filler()
                    for u in projqk_units(j + 1, "a"):
                        u()
                        clocks["pe"] += 1280
                    add_fillers(av2, 27 * (4 * j + 3))
                    add_fillers(
                        [lambda ot4=ot4: setattr(tposes, "oT2", transposes2(ot4))], 0
                    )
                    phase(feed_full(j + 1, 0))
                    add_fillers(projqk_units(j + 1, "b"), 1280)
                    add_fillers([mark(j + 1)], 0)
                    add_fillers(projv_units(j + 1), 480)
                    while fillers and tposes.oT2 is None:
                        emit_filler()
                    oT01, oT2 = tposes.oT01, tposes.oT2
                    tposes.oT01 = tposes.oT2 = None
                    add_fillers(
                        yproj_units(j, oT01, oT2, (nc.vector, nc.vector, nc.vector)),
                        426,
                    )
                else:
                    # ---- last block: drain fillers first so the h0/h1
                    # transposes are in flight, process head 2 per
                    # query-subtile, and run the output projection in two
                    # accumulation stages: h0/h1 contraction mid-loop, h2
                    # contraction + copies after the final transpose
                    while fillers:
                        emit_filler()
                    et_t = ets.pop((j, 2))
                    oq_t = oq_pool.tile([128, 4, 128], FP32)
                    oT2 = oT_pool.tile([128, SB], BF16, name="oT2")
                    jsp4 = slice(j * SB, (j + 1) * SB)
                    spA = sp_pool.tile([128, 2 * SB], FP32, name="sp")
                    spB = sp_pool.tile([128, 2 * SB], FP32, name="sp")
                    pjA = pj_pool.tile([128, SB], FP32, name="pj")
                    pjB = pj_pool.tile([128, SB], FP32, name="pj")
                    yps = [
                        spA[:, 0:SB], spA[:, SB : 2 * SB],
                        spB[:, 0:SB], spB[:, SB : 2 * SB],
                        pjA[:, :], pjB[:, :],
                    ]
                    for qs in range(4):
                        tq = 4 * j + qs + 1
                        for t in range(tq):
                            nc.tensor.matmul(
                                oq_t[:, qs, 0:65],
                                lhsT=av_lhsT(et_t, j, t, qs),
                                rhs=v65_sb[:, t, 2, :],
                                start=(t == 0),
                                stop=(t == tq - 1),
                            )
                        nc.vector.reciprocal(
                            rc_t[:, 8 + qs : 9 + qs], oq_t[:, qs, 64:65]
                        )
                        nc.vector.tensor_mul(
                            ot4[:, qs, 2, :],
                            oq_t[:, qs, 0:64],
                            rc_t[:, 8 + qs : 9 + qs].to_broadcast((128, 64)),
                        )
                        eng = nc.scalar if qs % 2 else nc.sync
                        eng.dma_start_transpose(
                            out=oT2[:, qs * 128 : (qs + 1) * 128],
                            in_=ot4[:, qs, 2:4, :],
                        )
                        if qs == 1:
                            # h0/h1 contraction stage: oT01 is ready by now
                            oT01 = tposes.oT01
                            for dt in range(KC):
                                nc.tensor.matmul(
                                    yps[dt],
                                    lhsT=wo0_sb[:, dt * 128 : (dt + 1) * 128],
                                    rhs=oT01,
                                    start=True,
                                    stop=False,
                                )
                    copy_engs = (nc.vector, nc.scalar, nc.vector)
                    for dt in range(KC):
                        nc.tensor.matmul(
                            yps[dt],
                            lhsT=wo1_sb[:, dt * 128 : (dt + 1) * 128],
                            rhs=oT2[0:64, :],
                            start=False,
                            stop=True,
                        )
                        yt = yt_pool.tile([128, SB], BF16, name="yt")
                        eng = copy_engs[dt % 3]
                        if eng is nc.scalar:
                            eng.copy(yt, yps[dt])
                        else:
                            eng.tensor_copy(yt, yps[dt])
                        (nc.sync if dt % 2 == 0 else nc.scalar).dma_start(
                            out=yT_d.ap()[dt * 128 : (dt + 1) * 128, jsp4], in_=yt
                        )
                    tposes.oT01 = None
            while fillers:
                emit_filler()
        else:
            run(projqk_units(0))
            for j in range(NJ):
                run(projv_units(j))
                if j + 1 < NJ:
                    run(projqk_units(j + 1))
            yprev = []
            for j in range(NJ):
                rc_t = rc_pool.tile([128, HG * 4], FP32)
                ot4 = ot_pool.tile([128, 4, 4, 64], BF16)
                nc.vector.memset(ot4[:, :, 3, :], 0.0)
                ets = [
                    et_pool.tile([128, NT, SB], BF16, name="et") for _ in range(HG)
                ]
                interleave(scores_full_units(j, 0, ets[0]), yprev)
                interleave(
                    scores_full_units(j, 1, ets[1]),
                    attnv_units(j, 0, ets[0], rc_t, ot4),
                )
                interleave(
                    scores_full_units(j, 2, ets[2]),
                    attnv_units(j, 1, ets[1], rc_t, ot4),
                )
                run(attnv_units(j, 2, ets[2], rc_t, ot4))
                oT01 = transposes01(ot4)
                oT2 = transposes2(ot4)
                engines = (nc.vector, nc.vector, nc.vector)
                yprev = list(yproj_units(j, oT01, oT2, engines))
                for u in yprev:
                    u()
                yprev = []

    nc.finalize()
    return nc


_NC_CACHE: dict[bool, object] = {}


def get_nc(causal: bool):
    if causal not in _NC_CACHE:
        _NC_CACHE[causal] = build_nc(causal)
    return _NC_CACHE[causal]


def _bf16(a):
    import ml_dtypes

    return np.asarray(a, np.float32).astype(ml_dtypes.bfloat16)


def _chunked(wT):
    """[768, N] -> [128, 6, N] with chunk c = rows 128c..128c+127."""
    n = wT.shape[1]
    return np.ascontiguousarray(wT.reshape(KC, 128, n).transpose(1, 0, 2))


def _make_cm4():
    # packed relative triangle masks [tri512 | tri384 | tri256 | tri128]:
    # tri(n)[p, c] = 1.0 iff c >= p; diagonal tile u uses tri(512 - 128u)
    p = np.arange(128)[:, None]
    segs = []
    for n in (512, 384, 256, 128):
        c = np.arange(n)[None, :]
        segs.append((c >= p).astype(np.float32))
    return np.concatenate(segs, axis=1)


def make_in_maps(x, wq, bq, wk, bk, wv, bv, wo, bo):
    f32 = np.float32
    x = np.asarray(x, f32)
    cm4 = _bf16(_make_cm4())
    in_maps = []
    for core in range(NCORES):
        b, hg = divmod(core, NH // HG)
        hs = slice(hg * HD, (hg + 1) * HD)
        wqT = np.asarray(wq, f32)[hs, :].T  # [768, 192]
        wkT = np.asarray(wk, f32)[hs, :].T
        wqkT = np.concatenate(
            [wqT[:, 0:128], wkT[:, 0:128], wqT[:, 128:192], wkT[:, 128:192]], axis=1
        )
        bqc = np.asarray(bq, f32)[hs]
        bqg = np.zeros((128, 2), f32)
        bqg[:, 0] = bqc[0:128]
        bqg[0:64, 1] = bqc[128:192]
        woT = np.asarray(wo, f32)[:, hs].T  # [192, 768]
        in_maps.append(
            {
                "x6": _bf16(_chunked(np.ascontiguousarray(x[b].T))),
                "wqk": _bf16(_chunked(wqkT)),
                "wv6": _bf16(_chunked(np.asarray(wv, f32)[hs, :].T)),
                "wo0": _bf16(woT[0:128, :]),
                "wo1": _bf16(woT[128:192, :]),
                "bqg": bqg,
                "cm4": cm4,
            }
        )
    return in_maps


def combine_outputs(results, wo, bv, bo):
    y = np.empty((B, S, D), np.float32)
    ng = NH // HG
    extra = (np.asarray(wo, np.float32) @ np.asarray(bv, np.float32)) + np.asarray(
        bo, np.float32
    )
    for b in range(B):
        acc = results[b * ng]["yT"].astype(np.float32)
        for g in range(1, ng):
            acc = acc + results[b * ng + g]["yT"].astype(np.float32)
        y[b] = acc.T + extra[None, :]
    return y


def kernel(x, wq, bq, wk, bk, wv, bv, wo, bo, mask, _trace=False):
    from concourse.bass_utils import run_bass_kernel_spmd

    causal = bool(np.asarray(mask).item())
    nc = get_nc(causal)
    in_maps = make_in_maps(x, wq, bq, wk, bk, wv, bv, wo, bo)
    res = run_bass_kernel_spmd(nc, in_maps, list(range(NCORES)), trace=_trace)
    y = combine_outputs(res.results, wo, bv, bo)
    if _trace:
        return y, res
    return y
